# revision 29
# baseline (speedup 1.0000x reference)
"""MultiHeadSelfAttentionWithLagBias on 8 TRN2 NeuronCores.

Sharding: tensor-parallel over heads — 16 heads / 8 cores = 2 heads per
core. Each core computes QKV projections for its head slice (full x),
attention with the lag bias for its 2 heads over both batch elements,
and a partial output projection (its 128 rows of wo). Host sums the 8
partials and adds bo.

v2 design notes (vs the v1 baseline at ~494us):
  - bias applied MULTIPLICATIVELY after exp: exp(s+b) = exp(s)*exp(b).
    Host precomputes exp(lag_bias)[|lag_i-lag_j|] in bf16, pre-tiled to
    the exact DMA layout (4KB contiguous per partition per transfer).
    This moves the bias op off the f32-PSUM path (DVE 1x, ~1.2us) onto
    a bf16 SBUF*SBUF mul (DVE 2x mode, ~0.6us) downstream of the exp.
  - exp output, bias table, V, and the attention-weight matrix are all
    bf16 (sim rel-err 5.2e-3 vs 2e-2 gate); x and the QKV weights are
    bf16 too (halves the 16.8MB xT stream).
  - attention inner loop is ACT(exp)-bound (~1.0us/iter x 128 iters);
    score+PV matmuls, the bias mul, and the bias DMA all pipeline under
    it via double/triple-buffered pools.
  - output projection + normalize deferred to a post-phase (PSUM banks:
    attention needs all 8: 2x2-bank score tiles + 4x1-bank O accum).
  - softmax denominator via the ones-column trick in the PV matmul
    (row 64 of each O accumulator); reciprocal via the fast DVE approx
    (51 ULP, plenty here) instead of the 4us iterative reciprocal.
"""

import ml_dtypes
import numpy as np
from contextlib import ExitStack

import concourse.bass as bass
import concourse.bacc as bacc
import concourse.mybir as mybir
import concourse.tile as tile
from concourse.bass_utils import run_bass_kernel_spmd
from concourse.masks import make_identity

F32 = mybir.dt.float32
F32R = mybir.dt.float32r
BF16 = mybir.dt.bfloat16
AF = mybir.ActivationFunctionType

N_CORES = 8
B, S, D = 2, 2048, 1024
H, DK = 16, 64
TOK = B * S              # 4096
NQ = 512                 # q-chunk (matmul free dim / PSUM bank)
NQC = S // NQ            # 4 q-chunks per batch
NJ = S // 128            # 16 k-chunks per batch
DCH = D // 128           # 8 contraction chunks

# Set by test.py for profiling; harness leaves these untouched.
TRACE = False
TRACE_DIR = None
DEBUG = False

_CACHED_NC = None


def _body(ctx: ExitStack, tc, aps):
    nc = tc.nc
    xT, wq, wk, wv, bq, bk, bv, wo, EB0, EB1, out = (
        aps["xT"], aps["wq"], aps["wk"], aps["wv"], aps["bq"], aps["bk"],
        aps["bv"], aps["wo"], aps["EB0"], aps["EB1"], aps["out"])
    EBh = [EB0, EB1]

    const = ctx.enter_context(tc.tile_pool(name="const", bufs=1))
    persist = ctx.enter_context(tc.tile_pool(name="persist", bufs=1))

    # ---- constants ----
    ident = const.tile([128, 128], F32, tag="id")
    make_identity(nc, ident[:])
    w_sb = {}
    for name, ap in (("q", wq), ("k", wk), ("v", wv)):
        t = const.tile([128, DCH, 128], BF16, tag=f"w{name}")
        nc.sync.dma_start(t[:], ap.rearrange("(c p) m -> p c m", p=128))
        w_sb[name] = t
    b_sb = {}
    for name, ap in (("q", bq), ("k", bk), ("v", bv)):
        t = const.tile([128, 1], F32, tag=f"b{name}")
        nc.sync.dma_start(t[:], ap[:])
        b_sb[name] = t
    # stationary row of ones for broadcasting the softmax reciprocal
    ones_row = const.tile([1, 64], F32, tag="ones_row")
    nc.vector.memset(ones_row[:], 1.0)

    # ---- persistent activations ----
    QT = persist.tile([128, TOK], BF16, tag="QT")
    KT = persist.tile([128, TOK], BF16, tag="KT")
    Vb = persist.tile([128, TOK // 128, 130], BF16, tag="Vb")
    OT = [persist.tile([65, TOK], BF16, tag=f"OT{h}", name=f"OT{h}")
          for h in range(2)]
    den_sb = [persist.tile([1, TOK], F32, tag=f"den{h}", name=f"den{h}")
              for h in range(2)]
    rec = [persist.tile([1, TOK], F32, tag=f"rec{h}", name=f"rec{h}")
           for h in range(2)]

    # ones columns of V_ext (positions 64 and 129 of each 130-stripe);
    # staged via an f32 memset + ACT copy (memset on strided bf16 is
    # unreliable).
    ones_f32 = const.tile([128, 64], F32, tag="ones_f32")
    nc.vector.memset(ones_f32[:], 1.0)
    nc.scalar.copy(
        Vb[:].rearrange("p t (g x) -> p t g x", g=2)[:, :, :, 64:65],
        ones_f32[:].rearrange("p (t g x) -> p t g x", t=TOK // 128, g=2))

    # ---- phases 1-2: QKV projections + V transpose (scoped pools) ----
    with tc.tile_pool(name="xin", bufs=3) as xpool, \
         tc.tile_pool(name="vtp", bufs=1) as vtpool, \
         tc.tile_pool(name="pj", bufs=3, space="PSUM") as pjpool, \
         tc.tile_pool(name="pt", bufs=2, space="PSUM") as ptpool:
        VT = vtpool.tile([128, TOK], F32, tag="VT")
        xT_r = xT.rearrange("(c p) n -> p c n", p=128)
        for t in range(TOK // NQ):
            xt = xpool.tile([128, DCH, NQ], BF16, tag="x")
            nc.sync.dma_start(xt[:], xT_r[:, :, t * NQ:(t + 1) * NQ])
            for name, dst in (("q", QT), ("k", KT), ("v", VT)):
                ps = pjpool.tile([128, NQ], F32, tag="pj")
                for d in range(DCH):
                    nc.tensor.matmul(ps[:], w_sb[name][:, d, :], xt[:, d, :],
                                     start=(d == 0), stop=(d == DCH - 1))
                nc.vector.tensor_scalar_add(
                    dst[:, t * NQ:(t + 1) * NQ], ps[:], b_sb[name][:])
            # V transpose for this token chunk (4 x 128-tok tiles)
            for u in range(t * 4, t * 4 + 4):
                pt = ptpool.tile([128, 128], F32, tag="pt")
                nc.tensor.transpose(pt[:], VT[:, u * 128:(u + 1) * 128],
                                    ident[:])
                nc.scalar.copy(
                    Vb[:, u, :].rearrange("p (g x) -> p g x", g=2)[:, :, 0:64],
                    pt[:].rearrange("p (g x) -> p g x", g=2))

    # ---- phase 3: attention (ACT-bound pipeline) ----
    with tc.tile_pool(name="eb", bufs=3) as ebpool, \
         tc.tile_pool(name="pr", bufs=3) as prpool, \
         tc.tile_pool(name="pe", bufs=3) as pepool, \
         tc.tile_pool(name="sp", bufs=2, space="PSUM") as spool, \
         tc.tile_pool(name="op", bufs=4, space="PSUM") as opool:
        # flattened (qc, jq) stream with bias-DMA prefetch across qc
        # boundaries; O-accumulator drains staggered into the next group's
        # iterations so they never head-block the DVE queue.
        ebt_tiles = [None] * 17
        deferred = []

        def issue_eb(g):
            qcg, jqg = divmod(g, 4)
            t = ebpool.tile([128, 2, 4, NQ], BF16, tag="eb")
            for h in range(2):
                r = g * 128
                nc.sync.dma_start(
                    t[:, h],
                    EBh[h][r:r + 128, :].rearrange("p (i q) -> p i q", i=4))
            ebt_tiles[g] = t

        def drain_O(O_ps, hh, b, qc):
            q0 = b * S + qc * NQ
            sl = slice(q0, q0 + NQ)
            # stash unnormalized O^T + denominator row (row 64); issued
            # immediately — it gates reuse of the PSUM accumulator bank.
            nc.vector.tensor_copy(OT[hh][:, sl], O_ps[hh][b][:])

            def c2():
                # den must reach a partition-0 F32 tile before the fast
                # reciprocal (the custom DVE op misbehaves fed the
                # partition-64 row directly; PSUM reads cannot shift
                # partitions). SBUF->SBUF shifted copy from the OT row.
                nc.vector.tensor_copy(den_sb[hh][0:1, sl],
                                      OT[hh][64:65, sl])
                nc.vector.reciprocal_approx_fast(
                    rec[hh][0:1, sl], den_sb[hh][0:1, sl])

            deferred.append(c2)

        issue_eb(0)
        O_ps = None
        for g in range(16):
            qc, jq = divmod(g, 4)
            if g + 1 < 16:
                issue_eb(g + 1)
            if jq == 0:
                O_ps = [[opool.tile([65, NQ], F32, tag="O", name=f"O{h}{b}")
                         for b in range(2)] for h in range(2)]
            ebt = ebt_tiles[g]
            for b in range(2):
                q0 = b * S + qc * NQ
                for ji in range(4):
                    j = jq * 4 + ji
                    k0 = b * S + j * 128
                    sps = spool.tile([128, 2 * NQ], F32, tag="s")
                    for hh in range(2):
                        nc.tensor.matmul(
                            sps[:, hh * NQ:(hh + 1) * NQ],
                            KT[64 * hh:64 * hh + 64, k0:k0 + 128],
                            QT[64 * hh:64 * hh + 64, q0:q0 + NQ],
                            start=True, stop=True)
                    pr = prpool.tile([128, 2 * NQ], BF16, tag="pr")
                    nc.scalar.activation(pr[:], sps[:], AF.Exp)
                    pe = pepool.tile([128, 2 * NQ], BF16, tag="pe")
                    nc.vector.tensor_mul(
                        pe[:].rearrange("p (h q) -> p h q", h=2),
                        pr[:].rearrange("p (h q) -> p h q", h=2),
                        ebt[:, :, ji, :])
                    if deferred:
                        deferred.pop(0)()
                    for hh in range(2):
                        nc.tensor.matmul(
                            O_ps[hh][b][:],
                            Vb[:, b * NJ + j, 65 * hh:65 * hh + 65],
                            pe[:, hh * NQ:(hh + 1) * NQ],
                            start=(j == 0), stop=(j == NJ - 1))
                if jq == 3 and b == 0:
                    for hh in range(2):
                        drain_O(O_ps, hh, 0, qc)
            ebt_tiles[g] = None
            if jq == 3:
                for hh in range(2):
                    drain_O(O_ps, hh, 1, qc)
        for fn in deferred:
            fn()
        deferred.clear()

    # ---- phase 4: normalize (all chunks), then a dense out-proj stream ----
    # wo loaded here, off the startup critical path (fires during attention)
    wo0 = const.tile([64, D], BF16, tag="wo0")
    wo1 = const.tile([64, D], BF16, tag="wo1")
    nc.sync.dma_start(wo0[:], wo[0:64, :])
    nc.sync.dma_start(wo1[:], wo[64:128, :])
    with tc.tile_pool(name="rp", bufs=2, space="PSUM") as rpool, \
         tc.tile_pool(name="os", bufs=3, space="PSUM") as ospool, \
         tc.tile_pool(name="dr", bufs=4) as drpool:
        for c in range(TOK // NQ):
            sl = slice(c * NQ, (c + 1) * NQ)
            for h in range(2):
                R = rpool.tile([64, NQ], F32, tag="R")
                nc.tensor.matmul(R[:], ones_row[:], rec[h][0:1, sl],
                                 start=True, stop=True)
                nc.vector.tensor_mul(OT[h][0:64, sl],
                                     OT[h][0:64, sl], R[:])
        for u in range(TOK // 128):
            ops = ospool.tile([128, 2 * NQ], F32, tag="os")
            for half in range(2):
                osl = slice(half * NQ, (half + 1) * NQ)
                nc.tensor.matmul(ops[:, osl],
                                 OT[0][0:64, u * 128:(u + 1) * 128],
                                 wo0[:, osl], start=True, stop=False)
                nc.tensor.matmul(ops[:, osl],
                                 OT[1][0:64, u * 128:(u + 1) * 128],
                                 wo1[:, osl], start=False, stop=True)
            osb = drpool.tile([128, 2 * NQ], BF16, tag="dr")
            # alternate engines so the drain isn't serialized on one
            if u % 2 == 0:
                nc.scalar.copy(osb[:], ops[:])
            else:
                nc.vector.tensor_copy(osb[:], ops[:])
            nc.sync.dma_start(out[u * 128:(u + 1) * 128, :], osb[:])

    if "den_dbg" in aps:
        nc.sync.dma_start(aps["den_dbg"], den_sb[0][:])
        nc.sync.dma_start(aps["rec_dbg"], rec[0][0:1, :])


def build_program():
    nc = bacc.Bacc("TRN2", target_bir_lowering=False, debug=False,
                   enable_asserts=False, num_devices=N_CORES)
    aps = {}
    specs = [
        ("xT", (D, TOK), BF16), ("wq", (D, 128), BF16), ("wk", (D, 128), BF16),
        ("wv", (D, 128), BF16), ("bq", (128, 1), F32), ("bk", (128, 1), F32),
        ("bv", (128, 1), F32), ("wo", (128, D), BF16),
        ("EB0", (NQC * 4 * 128, 2048), BF16), ("EB1", (NQC * 4 * 128, 2048), BF16),
    ]
    for name, shape, dt in specs:
        aps[name] = nc.dram_tensor(name, shape, dt, kind="ExternalInput").ap()
    aps["out"] = nc.dram_tensor("out", (TOK, D), BF16,
                                kind="ExternalOutput").ap()
    if DEBUG:
        for nm in ("den_dbg", "rec_dbg"):
            aps[nm] = nc.dram_tensor(nm, (1, TOK), F32,
                                     kind="ExternalOutput").ap()
    with tile.TileContext(nc) as tc:
        with ExitStack() as ctx:
            _body(ctx, tc, aps)
    nc.compile()
    return nc


def _get_nc():
    global _CACHED_NC
    if _CACHED_NC is None:
        _CACHED_NC = build_program()
    return _CACHED_NC


def _host_prep(x, lag, wq, bq, wk, bk, wv, bv, wo, bo, lag_bias):
    x = np.asarray(x, dtype=np.float32)
    lag = np.asarray(lag).astype(np.int64)
    xT = np.ascontiguousarray(
        x.reshape(TOK, D).T.astype(ml_dtypes.bfloat16))
    ld = np.abs(lag[:, None] - lag[None, :]).astype(np.int64)
    lag_bias = np.asarray(lag_bias, dtype=np.float32)
    exp_lb = np.exp(lag_bias).astype(np.float32)
    scale = np.float32(1.0 / np.sqrt(DK))
    wq = np.asarray(wq, dtype=np.float32) * scale
    bq = np.asarray(bq, dtype=np.float32) * scale
    in_maps = []
    for c in range(N_CORES):
        sl = slice(c * 128, (c + 1) * 128)
        cm = {
            "xT": xT,
            "wq": np.ascontiguousarray(wq[:, sl].astype(ml_dtypes.bfloat16)),
            "wk": np.ascontiguousarray(
                np.asarray(wk, np.float32)[:, sl].astype(ml_dtypes.bfloat16)),
            "wv": np.ascontiguousarray(
                np.asarray(wv, np.float32)[:, sl].astype(ml_dtypes.bfloat16)),
            "bq": np.ascontiguousarray(bq[sl].reshape(128, 1)),
            "bk": np.ascontiguousarray(
                np.asarray(bk, np.float32)[sl].reshape(128, 1)),
            "bv": np.ascontiguousarray(
                np.asarray(bv, np.float32)[sl].reshape(128, 1)),
            "wo": np.ascontiguousarray(
                np.asarray(wo, np.float32)[sl, :].astype(ml_dtypes.bfloat16)),
        }
        for hh in range(2):
            # exp(bias) gathered, then pre-tiled so each (qc, jq) DMA
            # reads [128, 4KB-contiguous-per-partition]:
            #   row (qc*4+jq)*128 + p, col ji*512 + q
            #   maps to bias[k = (jq*4+ji)*128 + p, qpos = qc*512 + q]
            eb = exp_lb[2 * c + hh][ld]                       # (S_k, S_q)
            eb6 = eb.reshape(4, 4, 128, NQC, NQ).transpose(3, 0, 2, 1, 4)
            cm[f"EB{hh}"] = np.ascontiguousarray(
                eb6.reshape(NQC * 4 * 128, 2048).astype(ml_dtypes.bfloat16))
        in_maps.append(cm)
    return in_maps


def kernel(x, lag, wq, bq, wk, bk, wv, bv, wo, bo, lag_bias):
    nc = _get_nc()
    in_maps = _host_prep(x, lag, wq, bq, wk, bk, wv, bv, wo, bo, lag_bias)
    kwargs = {}
    if TRACE:
        kwargs = dict(trace=True, tmpdir=TRACE_DIR)
    res = run_bass_kernel_spmd(nc, in_maps, core_ids=list(range(N_CORES)),
                               **kwargs)
    if TRACE:
        print(f"HW exec time: {res.exec_time_ns} ns")
    total = res.results[0]["out"].astype(np.float32)
    for c in range(1, N_CORES):
        total += res.results[c]["out"].astype(np.float32)
    total += np.asarray(bo, dtype=np.float32)[None, :]
    return total.reshape(B, S, D)


# revision 40
# speedup vs baseline: 1.1204x; 1.1204x over previous
"""MultiHeadSelfAttentionWithLagBias on 8 TRN2 NeuronCores.

Sharding: tensor-parallel over heads — 16 heads / 8 cores = 2 heads per
core. Each core computes QKV projections for its head slice (full x),
attention with the lag bias for its 2 heads over both batch elements,
and a partial output projection (its 128 rows of wo). Host sums the 8
partials and adds bo.

v2 design notes (vs the v1 baseline at ~494us):
  - bias applied MULTIPLICATIVELY after exp: exp(s+b) = exp(s)*exp(b).
    Host precomputes exp(lag_bias)[|lag_i-lag_j|] in bf16, pre-tiled to
    the exact DMA layout (4KB contiguous per partition per transfer).
    This moves the bias op off the f32-PSUM path (DVE 1x, ~1.2us) onto
    a bf16 SBUF*SBUF mul (DVE 2x mode, ~0.6us) downstream of the exp.
  - exp output, bias table, V, and the attention-weight matrix are all
    bf16 (sim rel-err 5.2e-3 vs 2e-2 gate); x and the QKV weights are
    bf16 too (halves the 16.8MB xT stream).
  - attention inner loop is ACT(exp)-bound (~1.0us/iter x 128 iters);
    score+PV matmuls, the bias mul, and the bias DMA all pipeline under
    it via double/triple-buffered pools.
  - output projection + normalize deferred to a post-phase (PSUM banks:
    attention needs all 8: 2x2-bank score tiles + 4x1-bank O accum).
  - softmax denominator via the ones-column trick in the PV matmul
    (row 64 of each O accumulator); reciprocal via the fast DVE approx
    (51 ULP, plenty here) instead of the 4us iterative reciprocal.
"""

import ml_dtypes
import numpy as np
from contextlib import ExitStack

import concourse.bass as bass
import concourse.bacc as bacc
import concourse.mybir as mybir
import concourse.tile as tile
from concourse.bass_utils import run_bass_kernel_spmd
from concourse.masks import make_identity

F32 = mybir.dt.float32
F32R = mybir.dt.float32r
BF16 = mybir.dt.bfloat16
AF = mybir.ActivationFunctionType

N_CORES = 8
B, S, D = 2, 2048, 1024
H, DK = 16, 64
TOK = B * S              # 4096
NQ = 512                 # q-chunk (matmul free dim / PSUM bank)
NQC = S // NQ            # 4 q-chunks per batch
NJ = S // 128            # 16 k-chunks per batch
DCH = D // 128           # 8 contraction chunks

# Set by test.py for profiling; harness leaves these untouched.
TRACE = False
TRACE_DIR = None
DEBUG = False

_CACHED_NC = None


def _body(ctx: ExitStack, tc, aps):
    nc = tc.nc
    xT, wq, wk, wv, bq, bk, bv, wo, EB0, EB1, out = (
        aps["xT"], aps["wq"], aps["wk"], aps["wv"], aps["bq"], aps["bk"],
        aps["bv"], aps["wo"], aps["EB0"], aps["EB1"], aps["out"])
    EBh = [EB0, EB1]

    const = ctx.enter_context(tc.tile_pool(name="const", bufs=1))
    persist = ctx.enter_context(tc.tile_pool(name="persist", bufs=1))

    # ---- constants ----
    ident = const.tile([128, 128], F32, tag="id")
    make_identity(nc, ident[:])
    w_sb = {}
    for name, ap in (("q", wq), ("k", wk), ("v", wv)):
        t = const.tile([128, DCH, 128], BF16, tag=f"w{name}")
        nc.sync.dma_start(t[:], ap.rearrange("(c p) m -> p c m", p=128))
        w_sb[name] = t
    b_sb = {}
    for name, ap in (("q", bq), ("k", bk), ("v", bv)):
        t = const.tile([128, 1], F32, tag=f"b{name}")
        nc.sync.dma_start(t[:], ap[:])
        b_sb[name] = t
    # stationary row of ones at partition 64 for broadcasting the softmax
    # denominator (bf16 to match the OT rhs dtype; partition 64 to match
    # the OT den row's base partition)
    ones_row = const.tile([65, 64], BF16, tag="ones_row")
    nc.vector.memset(ones_row[64:65, :], 1.0)

    # ---- persistent activations ----
    QT = persist.tile([128, TOK], BF16, tag="QT")
    KT = persist.tile([128, TOK], BF16, tag="KT")
    Vb = persist.tile([128, TOK // 128, 130], BF16, tag="Vb")
    OT = [persist.tile([65, TOK], BF16, tag=f"OT{h}", name=f"OT{h}")
          for h in range(2)]

    # ones columns of V_ext (positions 64 and 129 of each 130-stripe);
    # staged via an f32 memset + ACT copy (memset on strided bf16 is
    # unreliable).
    ones_f32 = const.tile([128, 64], F32, tag="ones_f32")
    nc.vector.memset(ones_f32[:], 1.0)
    nc.scalar.copy(
        Vb[:].rearrange("p t (g x) -> p t g x", g=2)[:, :, :, 64:65],
        ones_f32[:].rearrange("p (t g x) -> p t g x", t=TOK // 128, g=2))

    # ---- phases 1-2: QKV projections + V transpose (scoped pools) ----
    with tc.tile_pool(name="xin", bufs=3) as xpool, \
         tc.tile_pool(name="vtp", bufs=1) as vtpool, \
         tc.tile_pool(name="pj", bufs=3, space="PSUM") as pjpool, \
         tc.tile_pool(name="pt", bufs=2, space="PSUM") as ptpool:
        VT = vtpool.tile([128, TOK], F32, tag="VT")
        xT_r = xT.rearrange("(c p) n -> p c n", p=128)
        for t in range(TOK // NQ):
            xt = xpool.tile([128, DCH, NQ], BF16, tag="x")
            nc.sync.dma_start(xt[:], xT_r[:, :, t * NQ:(t + 1) * NQ])
            for name, dst in (("q", QT), ("k", KT), ("v", VT)):
                ps = pjpool.tile([128, NQ], F32, tag="pj")
                for d in range(DCH):
                    nc.tensor.matmul(ps[:], w_sb[name][:, d, :], xt[:, d, :],
                                     start=(d == 0), stop=(d == DCH - 1))
                nc.vector.tensor_scalar_add(
                    dst[:, t * NQ:(t + 1) * NQ], ps[:], b_sb[name][:])
            # V transpose for this token chunk (4 x 128-tok tiles)
            for u in range(t * 4, t * 4 + 4):
                pt = ptpool.tile([128, 128], F32, tag="pt")
                nc.tensor.transpose(pt[:], VT[:, u * 128:(u + 1) * 128],
                                    ident[:])
                nc.scalar.copy(
                    Vb[:, u, :].rearrange("p (g x) -> p g x", g=2)[:, :, 0:64],
                    pt[:].rearrange("p (g x) -> p g x", g=2))

    # ---- phase 3: attention (ACT-bound pipeline) ----
    with tc.tile_pool(name="eb", bufs=3) as ebpool, \
         tc.tile_pool(name="pr", bufs=3) as prpool, \
         tc.tile_pool(name="pe", bufs=3) as pepool, \
         tc.tile_pool(name="sp", bufs=2, space="PSUM") as spool, \
         tc.tile_pool(name="op", bufs=4, space="PSUM") as opool:
        # flattened (qc, jq) stream with bias-DMA prefetch across qc
        # boundaries.
        ebt_tiles = [None] * 17

        def issue_eb(g):
            qcg, jqg = divmod(g, 4)
            t = ebpool.tile([128, 2, 4, NQ], BF16, tag="eb")
            for h in range(2):
                r = g * 128
                nc.sync.dma_start(
                    t[:, h],
                    EBh[h][r:r + 128, :].rearrange("p (i q) -> p i q", i=4))
            ebt_tiles[g] = t

        def drain_O(O_ps, hh, b, qc):
            q0 = b * S + qc * NQ
            sl = slice(q0, q0 + NQ)
            # stash unnormalized O^T + denominator row (row 64); issued
            # immediately — it gates reuse of the PSUM accumulator bank.
            nc.vector.tensor_copy(OT[hh][:, sl], O_ps[hh][b][:])

        issue_eb(0)
        O_ps = None
        for g in range(16):
            qc, jq = divmod(g, 4)
            if g + 1 < 16:
                issue_eb(g + 1)
            if jq == 0:
                O_ps = [[opool.tile([65, NQ], F32, tag="O", name=f"O{h}{b}")
                         for b in range(2)] for h in range(2)]
            ebt = ebt_tiles[g]
            for b in range(2):
                q0 = b * S + qc * NQ
                for ji in range(4):
                    j = jq * 4 + ji
                    k0 = b * S + j * 128
                    sps = spool.tile([128, 2 * NQ], F32, tag="s")
                    for hh in range(2):
                        nc.tensor.matmul(
                            sps[:, hh * NQ:(hh + 1) * NQ],
                            KT[64 * hh:64 * hh + 64, k0:k0 + 128],
                            QT[64 * hh:64 * hh + 64, q0:q0 + NQ],
                            start=True, stop=True)
                    pr = prpool.tile([128, 2 * NQ], BF16, tag="pr")
                    nc.scalar.activation(pr[:], sps[:], AF.Exp)
                    pe = pepool.tile([128, 2 * NQ], BF16, tag="pe")
                    nc.vector.tensor_mul(
                        pe[:].rearrange("p (h q) -> p h q", h=2),
                        pr[:].rearrange("p (h q) -> p h q", h=2),
                        ebt[:, :, ji, :])
                    for hh in range(2):
                        nc.tensor.matmul(
                            O_ps[hh][b][:],
                            Vb[:, b * NJ + j, 65 * hh:65 * hh + 65],
                            pe[:, hh * NQ:(hh + 1) * NQ],
                            start=(j == 0), stop=(j == NJ - 1))
                if jq == 3 and b == 0:
                    for hh in range(2):
                        drain_O(O_ps, hh, 0, qc)
            ebt_tiles[g] = None
            if jq == 3:
                for hh in range(2):
                    drain_O(O_ps, hh, 1, qc)

    # ---- phase 4: normalize (all chunks), then a dense out-proj stream ----
    # wo loaded here, off the startup critical path (fires during attention)
    wo0 = const.tile([64, D], BF16, tag="wo0")
    wo1 = const.tile([64, D], BF16, tag="wo1")
    nc.sync.dma_start(wo0[:], wo[0:64, :])
    nc.sync.dma_start(wo1[:], wo[64:128, :])
    with tc.tile_pool(name="rp", bufs=2, space="PSUM") as rpool, \
         tc.tile_pool(name="rb", bufs=3) as rbpool, \
         tc.tile_pool(name="os", bufs=3, space="PSUM") as ospool, \
         tc.tile_pool(name="dr", bufs=4) as drpool:
        NC = TOK // NQ
        rec_bc = [None] * NC

        def normalize(c):
            # broadcast the bf16 den row (OT row 64) to 64 partitions via a
            # K=1 matmul, take the fast reciprocal on all 64 partitions at
            # once (FD-bound, one op), then scale OT in place.
            sl = slice(c * NQ, (c + 1) * NQ)
            rb = rbpool.tile([64, 2, NQ], F32, tag="rb")
            for h in range(2):
                R = rpool.tile([64, NQ], F32, tag="R")
                nc.tensor.matmul(R[:], ones_row[64:65, :], OT[h][64:65, sl],
                                 start=True, stop=True)
                nc.vector.reciprocal_approx_fast(rb[:, h, :], R[:])
            for h in range(2):
                nc.vector.tensor_mul(OT[h][0:64, sl],
                                     OT[h][0:64, sl], rb[:, h, :])
            rec_bc[c] = rb

        normalize(0)
        normalize(1)
        for c in range(NC):
            if c + 2 < NC:
                normalize(c + 2)
            for u in range(4 * c, 4 * c + 4):
                ops = ospool.tile([128, 2 * NQ], F32, tag="os")
                for half in range(2):
                    osl = slice(half * NQ, (half + 1) * NQ)
                    nc.tensor.matmul(ops[:, osl],
                                     OT[0][0:64, u * 128:(u + 1) * 128],
                                     wo0[:, osl], start=True, stop=False)
                    nc.tensor.matmul(ops[:, osl],
                                     OT[1][0:64, u * 128:(u + 1) * 128],
                                     wo1[:, osl], start=False, stop=True)
                osb = drpool.tile([128, 2 * NQ], BF16, tag="dr")
                # alternate engines so the drain isn't serialized on one
                if u % 2 == 0:
                    nc.scalar.copy(osb[:], ops[:])
                else:
                    nc.vector.tensor_copy(osb[:], ops[:])
                nc.sync.dma_start(out[u * 128:(u + 1) * 128, :], osb[:])
            rec_bc[c] = None


def build_program():
    nc = bacc.Bacc("TRN2", target_bir_lowering=False, debug=False,
                   enable_asserts=False, num_devices=N_CORES)
    aps = {}
    specs = [
        ("xT", (D, TOK), BF16), ("wq", (D, 128), BF16), ("wk", (D, 128), BF16),
        ("wv", (D, 128), BF16), ("bq", (128, 1), F32), ("bk", (128, 1), F32),
        ("bv", (128, 1), F32), ("wo", (128, D), BF16),
        ("EB0", (NQC * 4 * 128, 2048), BF16), ("EB1", (NQC * 4 * 128, 2048), BF16),
    ]
    for name, shape, dt in specs:
        aps[name] = nc.dram_tensor(name, shape, dt, kind="ExternalInput").ap()
    aps["out"] = nc.dram_tensor("out", (TOK, D), BF16,
                                kind="ExternalOutput").ap()
    with tile.TileContext(nc) as tc:
        with ExitStack() as ctx:
            _body(ctx, tc, aps)
    nc.compile()
    return nc


def _get_nc():
    global _CACHED_NC
    if _CACHED_NC is None:
        _CACHED_NC = build_program()
    return _CACHED_NC


def _host_prep(x, lag, wq, bq, wk, bk, wv, bv, wo, bo, lag_bias):
    x = np.asarray(x, dtype=np.float32)
    lag = np.asarray(lag).astype(np.int64)
    xT = np.ascontiguousarray(
        x.reshape(TOK, D).T.astype(ml_dtypes.bfloat16))
    ld = np.abs(lag[:, None] - lag[None, :]).astype(np.int64)
    lag_bias = np.asarray(lag_bias, dtype=np.float32)
    exp_lb = np.exp(lag_bias).astype(np.float32)
    scale = np.float32(1.0 / np.sqrt(DK))
    wq = np.asarray(wq, dtype=np.float32) * scale
    bq = np.asarray(bq, dtype=np.float32) * scale
    in_maps = []
    for c in range(N_CORES):
        sl = slice(c * 128, (c + 1) * 128)
        cm = {
            "xT": xT,
            "wq": np.ascontiguousarray(wq[:, sl].astype(ml_dtypes.bfloat16)),
            "wk": np.ascontiguousarray(
                np.asarray(wk, np.float32)[:, sl].astype(ml_dtypes.bfloat16)),
            "wv": np.ascontiguousarray(
                np.asarray(wv, np.float32)[:, sl].astype(ml_dtypes.bfloat16)),
            "bq": np.ascontiguousarray(bq[sl].reshape(128, 1)),
            "bk": np.ascontiguousarray(
                np.asarray(bk, np.float32)[sl].reshape(128, 1)),
            "bv": np.ascontiguousarray(
                np.asarray(bv, np.float32)[sl].reshape(128, 1)),
            "wo": np.ascontiguousarray(
                np.asarray(wo, np.float32)[sl, :].astype(ml_dtypes.bfloat16)),
        }
        for hh in range(2):
            # exp(bias) gathered, then pre-tiled so each (qc, jq) DMA
            # reads [128, 4KB-contiguous-per-partition]:
            #   row (qc*4+jq)*128 + p, col ji*512 + q
            #   maps to bias[k = (jq*4+ji)*128 + p, qpos = qc*512 + q]
            eb = exp_lb[2 * c + hh][ld]                       # (S_k, S_q)
            eb6 = eb.reshape(4, 4, 128, NQC, NQ).transpose(3, 0, 2, 1, 4)
            cm[f"EB{hh}"] = np.ascontiguousarray(
                eb6.reshape(NQC * 4 * 128, 2048).astype(ml_dtypes.bfloat16))
        in_maps.append(cm)
    return in_maps


def kernel(x, lag, wq, bq, wk, bk, wv, bv, wo, bo, lag_bias):
    nc = _get_nc()
    in_maps = _host_prep(x, lag, wq, bq, wk, bk, wv, bv, wo, bo, lag_bias)
    kwargs = {}
    if TRACE:
        kwargs = dict(trace=True, tmpdir=TRACE_DIR)
    res = run_bass_kernel_spmd(nc, in_maps, core_ids=list(range(N_CORES)),
                               **kwargs)
    if TRACE:
        print(f"HW exec time: {res.exec_time_ns} ns")
    total = res.results[0]["out"].astype(np.float32)
    for c in range(1, N_CORES):
        total += res.results[c]["out"].astype(np.float32)
    total += np.asarray(bo, dtype=np.float32)[None, :]
    return total.reshape(B, S, D)


# revision 52
# speedup vs baseline: 1.3394x; 1.1954x over previous
"""MultiHeadSelfAttentionWithLagBias on 8 TRN2 NeuronCores.

Sharding: tensor-parallel over heads — 16 heads / 8 cores = 2 heads per
core. Each core computes QKV projections for its head slice (full x),
attention with the lag bias for its 2 heads over both batch elements,
and a partial output projection (its 128 rows of wo). Host sums the 8
partials and adds bo.

v2 design notes (vs the v1 baseline at ~494us):
  - bias applied MULTIPLICATIVELY after exp: exp(s+b) = exp(s)*exp(b).
    Host precomputes exp(lag_bias)[|lag_i-lag_j|] in bf16, pre-tiled to
    the exact DMA layout (4KB contiguous per partition per transfer).
    This moves the bias op off the f32-PSUM path (DVE 1x, ~1.2us) onto
    a bf16 SBUF*SBUF mul (DVE 2x mode, ~0.6us) downstream of the exp.
  - exp output, bias table, V, and the attention-weight matrix are all
    bf16 (sim rel-err 5.2e-3 vs 2e-2 gate); x and the QKV weights are
    bf16 too (halves the 16.8MB xT stream).
  - attention inner loop is ACT(exp)-bound (~1.0us/iter x 128 iters);
    score+PV matmuls, the bias mul, and the bias DMA all pipeline under
    it via double/triple-buffered pools.
  - output projection + normalize deferred to a post-phase (PSUM banks:
    attention needs all 8: 2x2-bank score tiles + 4x1-bank O accum).
  - softmax denominator via the ones-column trick in the PV matmul
    (row 64 of each O accumulator); reciprocal via the fast DVE approx
    (51 ULP, plenty here) instead of the 4us iterative reciprocal.
"""

import ml_dtypes
import numpy as np
from contextlib import ExitStack

import concourse.bass as bass
import concourse.bacc as bacc
import concourse.mybir as mybir
import concourse.tile as tile
from concourse.bass_utils import run_bass_kernel_spmd
from concourse.masks import make_identity

F32 = mybir.dt.float32
F32R = mybir.dt.float32r
BF16 = mybir.dt.bfloat16
AF = mybir.ActivationFunctionType

N_CORES = 8
B, S, D = 2, 2048, 1024
H, DK = 16, 64
TOK = B * S              # 4096
NQ = 512                 # q-chunk (matmul free dim / PSUM bank)
NQC = S // NQ            # 4 q-chunks per batch
NJ = S // 128            # 16 k-chunks per batch
DCH = D // 128           # 8 contraction chunks

# Set by test.py for profiling; harness leaves these untouched.
TRACE = False
TRACE_DIR = None
DEBUG = False

_CACHED_NC = None


def _body(ctx: ExitStack, tc, aps):
    nc = tc.nc
    xT, wq, wk, wv, bq, bk, bv, wo, EB0, EB1, out = (
        aps["xT"], aps["wq"], aps["wk"], aps["wv"], aps["bq"], aps["bk"],
        aps["bv"], aps["wo"], aps["EB0"], aps["EB1"], aps["out"])
    EBh = [EB0, EB1]

    const = ctx.enter_context(tc.tile_pool(name="const", bufs=1))
    persist = ctx.enter_context(tc.tile_pool(name="persist", bufs=1))

    # ---- constants ----
    ident = const.tile([128, 128], F32, tag="id")
    make_identity(nc, ident[:])
    w_sb = {}
    for name, ap in (("q", wq), ("k", wk), ("v", wv)):
        t = const.tile([128, DCH, 128], BF16, tag=f"w{name}")
        nc.sync.dma_start(t[:], ap.rearrange("(c p) m -> p c m", p=128))
        w_sb[name] = t
    b_sb = {}
    for name, ap in (("q", bq), ("k", bk), ("v", bv)):
        t = const.tile([128, 1], F32, tag=f"b{name}")
        nc.sync.dma_start(t[:], ap[:])
        b_sb[name] = t
    # stationary row of ones at partition 64 for broadcasting the softmax
    # denominator (bf16 to match the OT rhs dtype; partition 64 to match
    # the OT den row's base partition)
    ones_row = const.tile([65, 64], BF16, tag="ones_row")
    nc.vector.memset(ones_row[64:65, :], 1.0)

    # ---- persistent activations ----
    QT = persist.tile([128, TOK], BF16, tag="QT")
    KT = persist.tile([128, TOK], BF16, tag="KT")
    Vb = persist.tile([128, TOK // 128, 130], BF16, tag="Vb")
    OT = [persist.tile([65, TOK], BF16, tag=f"OT{h}", name=f"OT{h}")
          for h in range(2)]

    # ones columns of V_ext (positions 64 and 129 of each 130-stripe);
    # staged via an f32 memset + ACT copy (memset on strided bf16 is
    # unreliable).
    ones_f32 = const.tile([128, 64], F32, tag="ones_f32")
    nc.vector.memset(ones_f32[:], 1.0)
    nc.scalar.copy(
        Vb[:].rearrange("p t (g x) -> p t g x", g=2)[:, :, :, 64:65],
        ones_f32[:].rearrange("p (t g x) -> p t g x", t=TOK // 128, g=2))

    # bias tiles are prefetched from inside the projection loop, so the
    # pool opens early
    ebpool = ctx.enter_context(tc.tile_pool(name="eb", bufs=3))
    ebt_tiles = [None] * 17

    def issue_eb(g):
        t = ebpool.tile([128, 2, 4, NQ], BF16, tag="eb")
        for h in range(2):
            r = g * 128
            nc.sync.dma_start(
                t[:, h],
                EBh[h][r:r + 128, :].rearrange("p (i q) -> p i q", i=4))
        ebt_tiles[g] = t

    # ---- phases 1-2: QKV projections + V transpose (scoped pools) ----
    with tc.tile_pool(name="xin", bufs=3) as xpool, \
         tc.tile_pool(name="vtp", bufs=1) as vtpool, \
         tc.tile_pool(name="pj", bufs=3, space="PSUM") as pjpool, \
         tc.tile_pool(name="pt", bufs=2, space="PSUM") as ptpool:
        VT = vtpool.tile([128, TOK], F32, tag="VT")
        xT_r = xT.rearrange("(c p) n -> p c n", p=128)
        for t in range(TOK // NQ):
            xt = xpool.tile([128, DCH, NQ], BF16, tag="x")
            nc.sync.dma_start(xt[:], xT_r[:, :, t * NQ:(t + 1) * NQ])
            for name, dst in (("q", QT), ("k", KT), ("v", VT)):
                ps = pjpool.tile([128, NQ], F32, tag="pj")
                for d in range(DCH):
                    nc.tensor.matmul(ps[:], w_sb[name][:, d, :], xt[:, d, :],
                                     start=(d == 0), stop=(d == DCH - 1))
                nc.vector.tensor_scalar_add(
                    dst[:, t * NQ:(t + 1) * NQ], ps[:], b_sb[name][:])
            # V transpose for this token chunk (4 x 128-tok tiles)
            for u in range(t * 4, t * 4 + 4):
                pt = ptpool.tile([128, 128], F32, tag="pt")
                nc.tensor.transpose(pt[:], VT[:, u * 128:(u + 1) * 128],
                                    ident[:])
                nc.scalar.copy(
                    Vb[:, u, :].rearrange("p (g x) -> p g x", g=2)[:, :, 0:64],
                    pt[:].rearrange("p (g x) -> p g x", g=2))
            if t >= 6:
                issue_eb(t - 6)  # prefetch the first bias tiles

    # ---- phase 3: attention (ACT-bound pipeline) ----
    with tc.tile_pool(name="pr", bufs=3) as prpool, \
         tc.tile_pool(name="pe", bufs=3) as pepool, \
         tc.tile_pool(name="sp", bufs=2, space="PSUM") as spool, \
         tc.tile_pool(name="op", bufs=4, space="PSUM") as opool:

        def drain_O(O_ps, hh, b, qc):
            q0 = b * S + qc * NQ
            sl = slice(q0, q0 + NQ)
            # stash unnormalized O^T + denominator row (row 64); issued
            # immediately — it gates reuse of the PSUM accumulator bank.
            nc.vector.tensor_copy(OT[hh][:, sl], O_ps[hh][b][:])

        O_ps = None
        for g in range(16):
            qc, jq = divmod(g, 4)
            if g + 1 < 16 and g + 1 >= 2:
                issue_eb(g + 1)
            if jq == 0:
                O_ps = [[opool.tile([65, NQ], F32, tag="O", name=f"O{h}{b}")
                         for b in range(2)] for h in range(2)]
            ebt = ebt_tiles[g]
            for b in range(2):
                q0 = b * S + qc * NQ
                for ji in range(4):
                    j = jq * 4 + ji
                    k0 = b * S + j * 128
                    sps = spool.tile([128, 2 * NQ], F32, tag="s")
                    for hh in range(2):
                        nc.tensor.matmul(
                            sps[:, hh * NQ:(hh + 1) * NQ],
                            KT[64 * hh:64 * hh + 64, k0:k0 + 128],
                            QT[64 * hh:64 * hh + 64, q0:q0 + NQ],
                            start=True, stop=True)
                    pr = prpool.tile([128, 2 * NQ], BF16, tag="pr")
                    nc.scalar.activation(pr[:], sps[:], AF.Exp)
                    pe = pepool.tile([128, 2 * NQ], BF16, tag="pe")
                    nc.vector.tensor_mul(
                        pe[:].rearrange("p (h q) -> p h q", h=2),
                        pr[:].rearrange("p (h q) -> p h q", h=2),
                        ebt[:, :, ji, :])
                    for hh in range(2):
                        nc.tensor.matmul(
                            O_ps[hh][b][:],
                            Vb[:, b * NJ + j, 65 * hh:65 * hh + 65],
                            pe[:, hh * NQ:(hh + 1) * NQ],
                            start=(j == 0), stop=(j == NJ - 1))
                if jq == 3 and b == 0:
                    for hh in range(2):
                        drain_O(O_ps, hh, 0, qc)
            ebt_tiles[g] = None
            if jq == 3:
                for hh in range(2):
                    drain_O(O_ps, hh, 1, qc)

    # ---- phase 4: normalize (all chunks), then a dense out-proj stream ----
    # wo loaded here, off the startup critical path (fires during attention)
    wo0 = const.tile([64, D], BF16, tag="wo0")
    wo1 = const.tile([64, D], BF16, tag="wo1")
    nc.sync.dma_start(wo0[:], wo[0:64, :])
    nc.sync.dma_start(wo1[:], wo[64:128, :])
    with tc.tile_pool(name="rp", bufs=2, space="PSUM") as rpool, \
         tc.tile_pool(name="rb", bufs=3) as rbpool, \
         tc.tile_pool(name="os", bufs=3, space="PSUM") as ospool, \
         tc.tile_pool(name="dr", bufs=4) as drpool:
        NC = TOK // NQ
        rec_bc = [None] * NC

        def normalize(c):
            # broadcast the bf16 den row (OT row 64) to 64 partitions via a
            # K=1 matmul, take the fast reciprocal on all 64 partitions at
            # once (FD-bound, one op), then scale OT in place.
            sl = slice(c * NQ, (c + 1) * NQ)
            rb = rbpool.tile([64, 2, NQ], F32, tag="rb")
            for h in range(2):
                R = rpool.tile([64, NQ], F32, tag="R")
                nc.tensor.matmul(R[:], ones_row[64:65, :], OT[h][64:65, sl],
                                 start=True, stop=True)
                nc.vector.reciprocal_approx_fast(rb[:, h, :], R[:])
            for h in range(2):
                nc.vector.tensor_mul(OT[h][0:64, sl],
                                     OT[h][0:64, sl], rb[:, h, :])
            rec_bc[c] = rb

        normalize(0)
        normalize(1)
        for c in range(NC):
            if c + 2 < NC:
                normalize(c + 2)
            for u in range(4 * c, 4 * c + 4):
                ops = ospool.tile([128, 2 * NQ], F32, tag="os")
                for half in range(2):
                    osl = slice(half * NQ, (half + 1) * NQ)
                    nc.tensor.matmul(ops[:, osl],
                                     OT[0][0:64, u * 128:(u + 1) * 128],
                                     wo0[:, osl], start=True, stop=False)
                    nc.tensor.matmul(ops[:, osl],
                                     OT[1][0:64, u * 128:(u + 1) * 128],
                                     wo1[:, osl], start=False, stop=True)
                osb = drpool.tile([128, 2 * NQ], BF16, tag="dr")
                # alternate engines so the drain isn't serialized on one
                if u % 2 == 0:
                    nc.scalar.copy(osb[:], ops[:])
                else:
                    nc.vector.tensor_copy(osb[:], ops[:])
                nc.sync.dma_start(out[u * 128:(u + 1) * 128, :], osb[:])
            rec_bc[c] = None


def build_program():
    nc = bacc.Bacc("TRN2", target_bir_lowering=False, debug=False,
                   enable_asserts=False, num_devices=N_CORES)
    aps = {}
    specs = [
        ("xT", (D, TOK), BF16), ("wq", (D, 128), BF16), ("wk", (D, 128), BF16),
        ("wv", (D, 128), BF16), ("bq", (128, 1), F32), ("bk", (128, 1), F32),
        ("bv", (128, 1), F32), ("wo", (128, D), BF16),
        ("EB0", (NQC * 4 * 128, 2048), BF16), ("EB1", (NQC * 4 * 128, 2048), BF16),
    ]
    for name, shape, dt in specs:
        aps[name] = nc.dram_tensor(name, shape, dt, kind="ExternalInput").ap()
    aps["out"] = nc.dram_tensor("out", (TOK, D), BF16,
                                kind="ExternalOutput").ap()
    with tile.TileContext(nc) as tc:
        with ExitStack() as ctx:
            _body(ctx, tc, aps)
    nc.compile()
    return nc


def _get_nc():
    global _CACHED_NC
    if _CACHED_NC is None:
        _CACHED_NC = build_program()
    return _CACHED_NC


def _host_prep(x, lag, wq, bq, wk, bk, wv, bv, wo, bo, lag_bias):
    x = np.asarray(x, dtype=np.float32)
    lag = np.asarray(lag).astype(np.int64)
    xT = np.ascontiguousarray(
        x.reshape(TOK, D).T.astype(ml_dtypes.bfloat16))
    ld = np.abs(lag[:, None] - lag[None, :]).astype(np.int64)
    lag_bias = np.asarray(lag_bias, dtype=np.float32)
    exp_lb = np.exp(lag_bias).astype(np.float32)
    scale = np.float32(1.0 / np.sqrt(DK))
    wq = np.asarray(wq, dtype=np.float32) * scale
    bq = np.asarray(bq, dtype=np.float32) * scale
    in_maps = []
    for c in range(N_CORES):
        sl = slice(c * 128, (c + 1) * 128)
        cm = {
            "xT": xT,
            "wq": np.ascontiguousarray(wq[:, sl].astype(ml_dtypes.bfloat16)),
            "wk": np.ascontiguousarray(
                np.asarray(wk, np.float32)[:, sl].astype(ml_dtypes.bfloat16)),
            "wv": np.ascontiguousarray(
                np.asarray(wv, np.float32)[:, sl].astype(ml_dtypes.bfloat16)),
            "bq": np.ascontiguousarray(bq[sl].reshape(128, 1)),
            "bk": np.ascontiguousarray(
                np.asarray(bk, np.float32)[sl].reshape(128, 1)),
            "bv": np.ascontiguousarray(
                np.asarray(bv, np.float32)[sl].reshape(128, 1)),
            "wo": np.ascontiguousarray(
                np.asarray(wo, np.float32)[sl, :].astype(ml_dtypes.bfloat16)),
        }
        for hh in range(2):
            # exp(bias) gathered, then pre-tiled so each (qc, jq) DMA
            # reads [128, 4KB-contiguous-per-partition]:
            #   row (qc*4+jq)*128 + p, col ji*512 + q
            #   maps to bias[k = (jq*4+ji)*128 + p, qpos = qc*512 + q]
            eb = exp_lb[2 * c + hh][ld]                       # (S_k, S_q)
            eb6 = eb.reshape(4, 4, 128, NQC, NQ).transpose(3, 0, 2, 1, 4)
            cm[f"EB{hh}"] = np.ascontiguousarray(
                eb6.reshape(NQC * 4 * 128, 2048).astype(ml_dtypes.bfloat16))
        in_maps.append(cm)
    return in_maps


def kernel(x, lag, wq, bq, wk, bk, wv, bv, wo, bo, lag_bias):
    nc = _get_nc()
    in_maps = _host_prep(x, lag, wq, bq, wk, bk, wv, bv, wo, bo, lag_bias)
    kwargs = {}
    if TRACE:
        kwargs = dict(trace=True, tmpdir=TRACE_DIR)
    res = run_bass_kernel_spmd(nc, in_maps, core_ids=list(range(N_CORES)),
                               **kwargs)
    if TRACE:
        print(f"HW exec time: {res.exec_time_ns} ns")
    total = res.results[0]["out"].astype(np.float32)
    for c in range(1, N_CORES):
        total += res.results[c]["out"].astype(np.float32)
    total += np.asarray(bo, dtype=np.float32)[None, :]
    return total.reshape(B, S, D)


# revision 63
# speedup vs baseline: 1.3621x; 1.0169x over previous
"""MultiHeadSelfAttentionWithLagBias on 8 TRN2 NeuronCores.

Sharding: tensor-parallel over heads — 16 heads / 8 cores = 2 heads per
core. Each core computes QKV projections for its head slice (full x),
attention with the lag bias for its 2 heads over both batch elements,
and a partial output projection (its 128 rows of wo). Host sums the 8
partials and adds bo.

v2 design notes (vs the v1 baseline at ~494us):
  - bias applied MULTIPLICATIVELY after exp: exp(s+b) = exp(s)*exp(b).
    Host precomputes exp(lag_bias)[|lag_i-lag_j|] in bf16, pre-tiled to
    the exact DMA layout (4KB contiguous per partition per transfer).
    This moves the bias op off the f32-PSUM path (DVE 1x, ~1.2us) onto
    a bf16 SBUF*SBUF mul (DVE 2x mode, ~0.6us) downstream of the exp.
  - exp output, bias table, V, and the attention-weight matrix are all
    bf16 (sim rel-err 5.2e-3 vs 2e-2 gate); x and the QKV weights are
    bf16 too (halves the 16.8MB xT stream).
  - attention inner loop is ACT(exp)-bound (~1.0us/iter x 128 iters);
    score+PV matmuls, the bias mul, and the bias DMA all pipeline under
    it via double/triple-buffered pools.
  - output projection + normalize deferred to a post-phase (PSUM banks:
    attention needs all 8: 2x2-bank score tiles + 4x1-bank O accum).
  - softmax denominator via the ones-column trick in the PV matmul
    (row 64 of each O accumulator); reciprocal via the fast DVE approx
    (51 ULP, plenty here) instead of the 4us iterative reciprocal.
"""

import ml_dtypes
import numpy as np
from contextlib import ExitStack

import concourse.bass as bass
import concourse.bacc as bacc
import concourse.mybir as mybir
import concourse.tile as tile
from concourse.bass_utils import run_bass_kernel_spmd

F32 = mybir.dt.float32
F32R = mybir.dt.float32r
BF16 = mybir.dt.bfloat16
AF = mybir.ActivationFunctionType

N_CORES = 8
B, S, D = 2, 2048, 1024
H, DK = 16, 64
TOK = B * S              # 4096
NQ = 512                 # q-chunk (matmul free dim / PSUM bank)
NQC = S // NQ            # 4 q-chunks per batch
NJ = S // 128            # 16 k-chunks per batch
DCH = D // 128           # 8 contraction chunks

# Set by test.py for profiling; harness leaves these untouched.
TRACE = False
TRACE_DIR = None
DEBUG = False

_CACHED_NC = None


def _body(ctx: ExitStack, tc, aps):
    nc = tc.nc
    xT, wq, wk, wv, bq, bk, bv, wo, EB0, EB1, out = (
        aps["xT"], aps["wq"], aps["wk"], aps["wv"], aps["bq"], aps["bk"],
        aps["bv"], aps["wo"], aps["EB0"], aps["EB1"], aps["out"])
    EBh = [EB0, EB1]

    const = ctx.enter_context(tc.tile_pool(name="const", bufs=1))
    persist = ctx.enter_context(tc.tile_pool(name="persist", bufs=1))

    # ---- constants ----
    w_sb = {}
    for name, ap in (("q", wq), ("k", wk), ("v", wv)):
        t = const.tile([128, DCH, 128], BF16, tag=f"w{name}")
        nc.sync.dma_start(t[:], ap.rearrange("(c p) m -> p c m", p=128))
        w_sb[name] = t
    b_sb = {}
    for name, ap in (("q", bq), ("k", bk)):
        t = const.tile([128, 1], F32, tag=f"b{name}")
        nc.sync.dma_start(t[:], ap[:])
        b_sb[name] = t
    # bv enters the V^T matmul as a K=1 ones-row rank-1 update
    bv_row = const.tile([1, 128], BF16, tag="bv_row")
    nc.sync.dma_start(bv_row[:], bv[:])
    ones128 = const.tile([1, 128], BF16, tag="ones128")
    nc.vector.memset(ones128[:], 1.0)
    # stationary row of ones at partition 64 for broadcasting the softmax
    # denominator (bf16 to match the OT rhs dtype; partition 64 to match
    # the OT den row's base partition)
    ones_row = const.tile([65, 64], BF16, tag="ones_row")
    nc.vector.memset(ones_row[64:65, :], 1.0)

    # ---- persistent activations ----
    QT = persist.tile([128, TOK], BF16, tag="QT")
    KT = persist.tile([128, TOK], BF16, tag="KT")
    Vb = persist.tile([128, TOK // 128, 130], BF16, tag="Vb")
    OT = [persist.tile([65, TOK], BF16, tag=f"OT{h}", name=f"OT{h}")
          for h in range(2)]

    # ones columns of V_ext (positions 64 and 129 of each 130-stripe);
    # staged via an f32 memset + ACT copy (memset on strided bf16 is
    # unreliable).
    ones_f32 = const.tile([128, 64], F32, tag="ones_f32")
    nc.vector.memset(ones_f32[:], 1.0)
    nc.scalar.copy(
        Vb[:].rearrange("p t (g x) -> p t g x", g=2)[:, :, :, 64:65],
        ones_f32[:].rearrange("p (t g x) -> p t g x", t=TOK // 128, g=2))

    # bias tiles are prefetched from inside the projection loop, so the
    # pool opens early
    ebpool = ctx.enter_context(tc.tile_pool(name="eb", bufs=3))
    ebt_tiles = [None] * 17

    def issue_eb(g):
        t = ebpool.tile([128, 2, 4, NQ], BF16, tag="eb")
        for h in range(2):
            r = g * 128
            nc.sync.dma_start(
                t[:, h],
                EBh[h][r:r + 128, :].rearrange("p (i q) -> p i q", i=4))
        ebt_tiles[g] = t

    # ---- phases 1-2: QKV projections (V^T computed directly) ----
    with tc.tile_pool(name="xin", bufs=3) as xpool, \
         tc.tile_pool(name="pj", bufs=3, space="PSUM") as pjpool, \
         tc.tile_pool(name="vp", bufs=3, space="PSUM") as vppool:
        xT_r = xT.rearrange("(c p) n -> p c n", p=128)
        for t in range(TOK // NQ):
            xt = xpool.tile([128, DCH, NQ], BF16, tag="x")
            if t == 0:
                # split the first transfer so the first matmuls can start
                # while the rest of the chunk streams in
                nc.sync.dma_start(xt[:, 0:2, :], xT_r[:, 0:2, 0:NQ])
                nc.sync.dma_start(xt[:, 2:DCH, :], xT_r[:, 2:DCH, 0:NQ])
            else:
                nc.sync.dma_start(xt[:], xT_r[:, :, t * NQ:(t + 1) * NQ])
            for name, dst in (("q", QT), ("k", KT)):
                ps = pjpool.tile([128, NQ], F32, tag="pj")
                for d in range(DCH):
                    nc.tensor.matmul(ps[:], w_sb[name][:, d, :], xt[:, d, :],
                                     start=(d == 0), stop=(d == DCH - 1))
                nc.vector.tensor_scalar_add(
                    dst[:, t * NQ:(t + 1) * NQ], ps[:], b_sb[name][:])
            # V^T produced directly: lhsT = x chunk (tokens as the free
            # dim), rhs = wv -> out [128 tok, 128 vdims]; bv added as a
            # K=1 rank-1 ones-row update. No PE transposes needed.
            for u in range(t * 4, t * 4 + 4):
                tsl = slice((u % 4) * 128, (u % 4) * 128 + 128)
                vps = vppool.tile([128, 128], F32, tag="vp")
                for d in range(DCH):
                    nc.tensor.matmul(vps[:], xt[:, d, tsl],
                                     w_sb["v"][:, d, :],
                                     start=(d == 0), stop=False)
                nc.tensor.matmul(vps[:], ones128[:], bv_row[:],
                                 start=False, stop=True)
                nc.vector.tensor_copy(
                    Vb[:, u, :].rearrange("p (g x) -> p g x", g=2)[:, :, 0:64],
                    vps[:].rearrange("p (g x) -> p g x", g=2))
            if t >= 6:
                issue_eb(t - 6)  # prefetch the first bias tiles

    # ---- phase 3: attention (ACT-bound pipeline) ----
    with tc.tile_pool(name="pr", bufs=3) as prpool, \
         tc.tile_pool(name="pe", bufs=3) as pepool, \
         tc.tile_pool(name="sp", bufs=2, space="PSUM") as spool, \
         tc.tile_pool(name="op", bufs=4, space="PSUM") as opool:

        # O-accumulator drain copies are deferred and dribbled out one per
        # iteration (popped right after each bias-mul) so they never
        # head-block the DVE queue. A batch-b accumulator is re-allocated
        # one qc later, right before its batch's loop — by then its drain
        # has been issued during the preceding iterations (with a safety
        # flush at allocation).
        pending = {0: [], 1: []}

        def drain_O(O_ps, hh, b, qc):
            q0 = b * S + qc * NQ
            sl = slice(q0, q0 + NQ)
            tile_ref = O_ps[hh][b]
            pending[b].append(
                lambda: nc.vector.tensor_copy(OT[hh][:, sl], tile_ref[:]))

        def pop_one():
            for bb in (0, 1):
                if pending[bb]:
                    pending[bb].pop(0)()
                    return

        O_ps = [[None, None], [None, None]]
        for g in range(16):
            qc, jq = divmod(g, 4)
            if g + 1 < 16 and g + 1 >= 2:
                issue_eb(g + 1)
            ebt = ebt_tiles[g]
            for b in range(2):
                if jq == 0:
                    while pending[b]:
                        pending[b].pop(0)()
                    for h in range(2):
                        O_ps[h][b] = opool.tile([65, NQ], F32, tag="O",
                                                name=f"O{h}{b}")
                q0 = b * S + qc * NQ
                for ji in range(4):
                    j = jq * 4 + ji
                    k0 = b * S + j * 128
                    sps = spool.tile([128, 2 * NQ], F32, tag="s")
                    for hh in range(2):
                        nc.tensor.matmul(
                            sps[:, hh * NQ:(hh + 1) * NQ],
                            KT[64 * hh:64 * hh + 64, k0:k0 + 128],
                            QT[64 * hh:64 * hh + 64, q0:q0 + NQ],
                            start=True, stop=True)
                    pr = prpool.tile([128, 2 * NQ], BF16, tag="pr")
                    nc.scalar.activation(pr[:], sps[:], AF.Exp)
                    pe = pepool.tile([128, 2 * NQ], BF16, tag="pe")
                    nc.vector.tensor_mul(
                        pe[:].rearrange("p (h q) -> p h q", h=2),
                        pr[:].rearrange("p (h q) -> p h q", h=2),
                        ebt[:, :, ji, :])
                    pop_one()
                    for hh in range(2):
                        nc.tensor.matmul(
                            O_ps[hh][b][:],
                            Vb[:, b * NJ + j, 65 * hh:65 * hh + 65],
                            pe[:, hh * NQ:(hh + 1) * NQ],
                            start=(j == 0), stop=(j == NJ - 1))
                if jq == 3 and b == 0:
                    for hh in range(2):
                        drain_O(O_ps, hh, 0, qc)
            ebt_tiles[g] = None
            if jq == 3:
                for hh in range(2):
                    drain_O(O_ps, hh, 1, qc)
        for bb in (0, 1):
            while pending[bb]:
                pending[bb].pop(0)()

    # ---- phase 4: normalize (all chunks), then a dense out-proj stream ----
    # wo loaded here, off the startup critical path (fires during attention)
    wo0 = const.tile([64, D], BF16, tag="wo0")
    wo1 = const.tile([64, D], BF16, tag="wo1")
    nc.sync.dma_start(wo0[:], wo[0:64, :])
    nc.sync.dma_start(wo1[:], wo[64:128, :])
    with tc.tile_pool(name="rp", bufs=2, space="PSUM") as rpool, \
         tc.tile_pool(name="rb", bufs=3) as rbpool, \
         tc.tile_pool(name="os", bufs=3, space="PSUM") as ospool, \
         tc.tile_pool(name="dr", bufs=4) as drpool:
        NC = TOK // NQ
        rec_bc = [None] * NC

        def normalize(c):
            # broadcast the bf16 den row (OT row 64) to 64 partitions via a
            # K=1 matmul, take the fast reciprocal on all 64 partitions at
            # once (FD-bound, one op), then scale OT in place.
            sl = slice(c * NQ, (c + 1) * NQ)
            rb = rbpool.tile([64, 2, NQ], F32, tag="rb")
            for h in range(2):
                R = rpool.tile([64, NQ], F32, tag="R")
                nc.tensor.matmul(R[:], ones_row[64:65, :], OT[h][64:65, sl],
                                 start=True, stop=True)
                nc.vector.reciprocal_approx_fast(rb[:, h, :], R[:])
            for h in range(2):
                nc.vector.tensor_mul(OT[h][0:64, sl],
                                     OT[h][0:64, sl], rb[:, h, :])
            rec_bc[c] = rb

        normalize(0)
        normalize(1)
        for c in range(NC):
            if c + 2 < NC:
                normalize(c + 2)
            for u in range(4 * c, 4 * c + 4):
                ops = ospool.tile([128, 2 * NQ], F32, tag="os")
                for half in range(2):
                    osl = slice(half * NQ, (half + 1) * NQ)
                    nc.tensor.matmul(ops[:, osl],
                                     OT[0][0:64, u * 128:(u + 1) * 128],
                                     wo0[:, osl], start=True, stop=False)
                    nc.tensor.matmul(ops[:, osl],
                                     OT[1][0:64, u * 128:(u + 1) * 128],
                                     wo1[:, osl], start=False, stop=True)
                osb = drpool.tile([128, 2 * NQ], BF16, tag="dr")
                # alternate engines so the drain isn't serialized on one
                if u % 2 == 0:
                    nc.scalar.copy(osb[:], ops[:])
                else:
                    nc.vector.tensor_copy(osb[:], ops[:])
                nc.sync.dma_start(out[u * 128:(u + 1) * 128, :], osb[:])
            rec_bc[c] = None


def build_program():
    nc = bacc.Bacc("TRN2", target_bir_lowering=False, debug=False,
                   enable_asserts=False, num_devices=N_CORES)
    aps = {}
    specs = [
        ("xT", (D, TOK), BF16), ("wq", (D, 128), BF16), ("wk", (D, 128), BF16),
        ("wv", (D, 128), BF16), ("bq", (128, 1), F32), ("bk", (128, 1), F32),
        ("bv", (1, 128), BF16), ("wo", (128, D), BF16),
        ("EB0", (NQC * 4 * 128, 2048), BF16), ("EB1", (NQC * 4 * 128, 2048), BF16),
    ]
    for name, shape, dt in specs:
        aps[name] = nc.dram_tensor(name, shape, dt, kind="ExternalInput").ap()
    aps["out"] = nc.dram_tensor("out", (TOK, D), BF16,
                                kind="ExternalOutput").ap()
    with tile.TileContext(nc) as tc:
        with ExitStack() as ctx:
            _body(ctx, tc, aps)
    nc.compile()
    return nc


def _get_nc():
    global _CACHED_NC
    if _CACHED_NC is None:
        _CACHED_NC = build_program()
    return _CACHED_NC


def _host_prep(x, lag, wq, bq, wk, bk, wv, bv, wo, bo, lag_bias):
    x = np.asarray(x, dtype=np.float32)
    lag = np.asarray(lag).astype(np.int64)
    xT = np.ascontiguousarray(
        x.reshape(TOK, D).T.astype(ml_dtypes.bfloat16))
    ld = np.abs(lag[:, None] - lag[None, :]).astype(np.int64)
    lag_bias = np.asarray(lag_bias, dtype=np.float32)
    exp_lb = np.exp(lag_bias).astype(np.float32)
    scale = np.float32(1.0 / np.sqrt(DK))
    wq = np.asarray(wq, dtype=np.float32) * scale
    bq = np.asarray(bq, dtype=np.float32) * scale
    in_maps = []
    for c in range(N_CORES):
        sl = slice(c * 128, (c + 1) * 128)
        cm = {
            "xT": xT,
            "wq": np.ascontiguousarray(wq[:, sl].astype(ml_dtypes.bfloat16)),
            "wk": np.ascontiguousarray(
                np.asarray(wk, np.float32)[:, sl].astype(ml_dtypes.bfloat16)),
            "wv": np.ascontiguousarray(
                np.asarray(wv, np.float32)[:, sl].astype(ml_dtypes.bfloat16)),
            "bq": np.ascontiguousarray(bq[sl].reshape(128, 1)),
            "bk": np.ascontiguousarray(
                np.asarray(bk, np.float32)[sl].reshape(128, 1)),
            "bv": np.ascontiguousarray(
                np.asarray(bv, np.float32)[sl].reshape(1, 128)
                .astype(ml_dtypes.bfloat16)),
            "wo": np.ascontiguousarray(
                np.asarray(wo, np.float32)[sl, :].astype(ml_dtypes.bfloat16)),
        }
        for hh in range(2):
            # exp(bias) gathered, then pre-tiled so each (qc, jq) DMA
            # reads [128, 4KB-contiguous-per-partition]:
            #   row (qc*4+jq)*128 + p, col ji*512 + q
            #   maps to bias[k = (jq*4+ji)*128 + p, qpos = qc*512 + q]
            eb = exp_lb[2 * c + hh][ld]                       # (S_k, S_q)
            eb6 = eb.reshape(4, 4, 128, NQC, NQ).transpose(3, 0, 2, 1, 4)
            cm[f"EB{hh}"] = np.ascontiguousarray(
                eb6.reshape(NQC * 4 * 128, 2048).astype(ml_dtypes.bfloat16))
        in_maps.append(cm)
    return in_maps


def kernel(x, lag, wq, bq, wk, bk, wv, bv, wo, bo, lag_bias):
    nc = _get_nc()
    in_maps = _host_prep(x, lag, wq, bq, wk, bk, wv, bv, wo, bo, lag_bias)
    kwargs = {}
    if TRACE:
        kwargs = dict(trace=True, tmpdir=TRACE_DIR)
    res = run_bass_kernel_spmd(nc, in_maps, core_ids=list(range(N_CORES)),
                               **kwargs)
    if TRACE:
        print(f"HW exec time: {res.exec_time_ns} ns")
    total = res.results[0]["out"].astype(np.float32)
    for c in range(1, N_CORES):
        total += res.results[c]["out"].astype(np.float32)
    total += np.asarray(bo, dtype=np.float32)[None, :]
    return total.reshape(B, S, D)


# revision 68
# speedup vs baseline: 1.3729x; 1.0079x over previous
"""MultiHeadSelfAttentionWithLagBias on 8 TRN2 NeuronCores.

Sharding: tensor-parallel over heads — 16 heads / 8 cores = 2 heads per
core. Each core computes QKV projections for its head slice (full x),
attention with the lag bias for its 2 heads over both batch elements,
and a partial output projection (its 128 rows of wo). Host sums the 8
partials and adds bo.

v2 design notes (vs the v1 baseline at ~494us):
  - bias applied MULTIPLICATIVELY after exp: exp(s+b) = exp(s)*exp(b).
    Host precomputes exp(lag_bias)[|lag_i-lag_j|] in bf16, pre-tiled to
    the exact DMA layout (4KB contiguous per partition per transfer).
    This moves the bias op off the f32-PSUM path (DVE 1x, ~1.2us) onto
    a bf16 SBUF*SBUF mul (DVE 2x mode, ~0.6us) downstream of the exp.
  - exp output, bias table, V, and the attention-weight matrix are all
    bf16 (sim rel-err 5.2e-3 vs 2e-2 gate); x and the QKV weights are
    bf16 too (halves the 16.8MB xT stream).
  - attention inner loop is ACT(exp)-bound (~1.0us/iter x 128 iters);
    score+PV matmuls, the bias mul, and the bias DMA all pipeline under
    it via double/triple-buffered pools.
  - output projection + normalize deferred to a post-phase (PSUM banks:
    attention needs all 8: 2x2-bank score tiles + 4x1-bank O accum).
  - softmax denominator via the ones-column trick in the PV matmul
    (row 64 of each O accumulator); reciprocal via the fast DVE approx
    (51 ULP, plenty here) instead of the 4us iterative reciprocal.
"""

import ml_dtypes
import numpy as np
from contextlib import ExitStack

import concourse.bass as bass
import concourse.bacc as bacc
import concourse.mybir as mybir
import concourse.tile as tile
from concourse.bass_utils import run_bass_kernel_spmd
from concourse.masks import make_identity

F32 = mybir.dt.float32
F32R = mybir.dt.float32r
BF16 = mybir.dt.bfloat16
AF = mybir.ActivationFunctionType

N_CORES = 8
B, S, D = 2, 2048, 1024
H, DK = 16, 64
TOK = B * S              # 4096
NQ = 512                 # q-chunk (matmul free dim / PSUM bank)
NQC = S // NQ            # 4 q-chunks per batch
NJ = S // 128            # 16 k-chunks per batch
DCH = D // 128           # 8 contraction chunks

# Set by test.py for profiling; harness leaves these untouched.
TRACE = False
TRACE_DIR = None
DEBUG = False

_CACHED_NC = None


def _body(ctx: ExitStack, tc, aps):
    nc = tc.nc
    xT, wq, wk, wv, bq, bk, bv, wo, EB0, EB1, out = (
        aps["xT"], aps["wq"], aps["wk"], aps["wv"], aps["bq"], aps["bk"],
        aps["bv"], aps["wo"], aps["EB0"], aps["EB1"], aps["out"])
    EBh = [EB0, EB1]

    const = ctx.enter_context(tc.tile_pool(name="const", bufs=1))
    persist = ctx.enter_context(tc.tile_pool(name="persist", bufs=1))

    # ---- constants ----
    w_sb = {}
    for name, ap in (("q", wq), ("k", wk), ("v", wv)):
        t = const.tile([128, DCH, 128], BF16, tag=f"w{name}")
        nc.sync.dma_start(t[:], ap.rearrange("(c p) m -> p c m", p=128))
        w_sb[name] = t
    b_sb = {}
    for name, ap in (("q", bq), ("k", bk), ("v", bv)):
        t = const.tile([128, 1], F32, tag=f"b{name}")
        nc.sync.dma_start(t[:], ap[:])
        b_sb[name] = t
    ident = const.tile([128, 128], F32, tag="id")
    make_identity(nc, ident[:])
    # stationary row of ones at partition 64 for broadcasting the softmax
    # denominator (bf16 to match the OT rhs dtype; partition 64 to match
    # the OT den row's base partition)
    ones_row = const.tile([65, 64], BF16, tag="ones_row")
    nc.vector.memset(ones_row[64:65, :], 1.0)

    # ---- persistent activations ----
    QT = persist.tile([128, TOK], BF16, tag="QT")
    KT = persist.tile([128, TOK], BF16, tag="KT")
    Vb = persist.tile([128, TOK // 128, 130], BF16, tag="Vb")
    OT = [persist.tile([65, TOK], BF16, tag=f"OT{h}", name=f"OT{h}")
          for h in range(2)]

    # ones columns of V_ext (positions 64 and 129 of each 130-stripe);
    # staged via an f32 memset + ACT copy (memset on strided bf16 is
    # unreliable).
    ones_f32 = const.tile([128, 64], F32, tag="ones_f32")
    nc.vector.memset(ones_f32[:], 1.0)
    nc.scalar.copy(
        Vb[:].rearrange("p t (g x) -> p t g x", g=2)[:, :, :, 64:65],
        ones_f32[:].rearrange("p (t g x) -> p t g x", t=TOK // 128, g=2))

    # bias tiles are prefetched from inside the projection loop, so the
    # pool opens early
    ebpool = ctx.enter_context(tc.tile_pool(name="eb", bufs=3))
    ebt_tiles = [None] * 17

    def issue_eb(g):
        t = ebpool.tile([128, 2, 4, NQ], BF16, tag="eb")
        for h in range(2):
            r = g * 128
            nc.sync.dma_start(
                t[:, h],
                EBh[h][r:r + 128, :].rearrange("p (i q) -> p i q", i=4))
        ebt_tiles[g] = t

    # ---- phases 1-2: QKV projections + V transpose (scoped pools) ----
    with tc.tile_pool(name="xin", bufs=3) as xpool, \
         tc.tile_pool(name="vtp", bufs=1) as vtpool, \
         tc.tile_pool(name="pj", bufs=3, space="PSUM") as pjpool, \
         tc.tile_pool(name="pt", bufs=2, space="PSUM") as ptpool:
        VT = vtpool.tile([128, TOK], F32, tag="VT")
        xT_r = xT.rearrange("(c p) n -> p c n", p=128)
        for t in range(TOK // NQ):
            xt = xpool.tile([128, DCH, NQ], BF16, tag="x")
            if t == 0:
                # split the first transfer so the first matmuls can start
                # while the rest of the chunk streams in
                nc.sync.dma_start(xt[:, 0:2, :], xT_r[:, 0:2, 0:NQ])
                nc.sync.dma_start(xt[:, 2:DCH, :], xT_r[:, 2:DCH, 0:NQ])
            else:
                nc.sync.dma_start(xt[:], xT_r[:, :, t * NQ:(t + 1) * NQ])
            for name, dst in (("q", QT), ("k", KT), ("v", VT)):
                ps = pjpool.tile([128, NQ], F32, tag="pj")
                for d in range(DCH):
                    nc.tensor.matmul(ps[:], w_sb[name][:, d, :], xt[:, d, :],
                                     start=(d == 0), stop=(d == DCH - 1))
                nc.vector.tensor_scalar_add(
                    dst[:, t * NQ:(t + 1) * NQ], ps[:], b_sb[name][:])
            # V transpose for this token chunk (4 x 128-tok tiles)
            for u in range(t * 4, t * 4 + 4):
                pt = ptpool.tile([128, 128], F32, tag="pt")
                nc.tensor.transpose(pt[:], VT[:, u * 128:(u + 1) * 128],
                                    ident[:])
                nc.scalar.copy(
                    Vb[:, u, :].rearrange("p (g x) -> p g x", g=2)[:, :, 0:64],
                    pt[:].rearrange("p (g x) -> p g x", g=2))
            if t >= 6:
                issue_eb(t - 6)  # prefetch the first bias tiles

    # ---- phase 3: attention (ACT-bound pipeline) ----
    with tc.tile_pool(name="pr", bufs=3) as prpool, \
         tc.tile_pool(name="pe", bufs=3) as pepool, \
         tc.tile_pool(name="sp", bufs=2, space="PSUM") as spool, \
         tc.tile_pool(name="op", bufs=4, space="PSUM") as opool:

        # O-accumulator drain copies are deferred and dribbled out one per
        # iteration (popped right after each bias-mul) so they never
        # head-block the DVE queue. A batch-b accumulator is re-allocated
        # one qc later, right before its batch's loop — by then its drain
        # has been issued during the preceding iterations (with a safety
        # flush at allocation).
        pending = {0: [], 1: []}

        def drain_O(O_ps, hh, b, qc):
            q0 = b * S + qc * NQ
            sl = slice(q0, q0 + NQ)
            tile_ref = O_ps[hh][b]
            pending[b].append(
                lambda: nc.vector.tensor_copy(OT[hh][:, sl], tile_ref[:]))

        def pop_one():
            for bb in (0, 1):
                if pending[bb]:
                    pending[bb].pop(0)()
                    return

        O_ps = [[None, None], [None, None]]
        for g in range(16):
            qc, jq = divmod(g, 4)
            if g + 1 < 16 and g + 1 >= 2:
                issue_eb(g + 1)
            ebt = ebt_tiles[g]
            for b in range(2):
                if jq == 0:
                    while pending[b]:
                        pending[b].pop(0)()
                    for h in range(2):
                        O_ps[h][b] = opool.tile([65, NQ], F32, tag="O",
                                                name=f"O{h}{b}")
                q0 = b * S + qc * NQ
                for ji in range(4):
                    j = jq * 4 + ji
                    k0 = b * S + j * 128
                    sps = spool.tile([128, 2 * NQ], F32, tag="s")
                    for hh in range(2):
                        nc.tensor.matmul(
                            sps[:, hh * NQ:(hh + 1) * NQ],
                            KT[64 * hh:64 * hh + 64, k0:k0 + 128],
                            QT[64 * hh:64 * hh + 64, q0:q0 + NQ],
                            start=True, stop=True)
                    pr = prpool.tile([128, 2 * NQ], BF16, tag="pr")
                    nc.scalar.activation(pr[:], sps[:], AF.Exp)
                    pe = pepool.tile([128, 2 * NQ], BF16, tag="pe")
                    nc.vector.tensor_mul(
                        pe[:].rearrange("p (h q) -> p h q", h=2),
                        pr[:].rearrange("p (h q) -> p h q", h=2),
                        ebt[:, :, ji, :])
                    pop_one()
                    for hh in range(2):
                        nc.tensor.matmul(
                            O_ps[hh][b][:],
                            Vb[:, b * NJ + j, 65 * hh:65 * hh + 65],
                            pe[:, hh * NQ:(hh + 1) * NQ],
                            start=(j == 0), stop=(j == NJ - 1))
                if jq == 3 and b == 0:
                    for hh in range(2):
                        drain_O(O_ps, hh, 0, qc)
            ebt_tiles[g] = None
            if jq == 3:
                for hh in range(2):
                    drain_O(O_ps, hh, 1, qc)
        for bb in (0, 1):
            while pending[bb]:
                pending[bb].pop(0)()

    # ---- phase 4: normalize (all chunks), then a dense out-proj stream ----
    # wo loaded here, off the startup critical path (fires during attention)
    wo0 = const.tile([64, D], BF16, tag="wo0")
    wo1 = const.tile([64, D], BF16, tag="wo1")
    nc.sync.dma_start(wo0[:], wo[0:64, :])
    nc.sync.dma_start(wo1[:], wo[64:128, :])
    with tc.tile_pool(name="rp", bufs=2, space="PSUM") as rpool, \
         tc.tile_pool(name="rb", bufs=3) as rbpool, \
         tc.tile_pool(name="os", bufs=3, space="PSUM") as ospool, \
         tc.tile_pool(name="dr", bufs=4) as drpool:
        NC = TOK // NQ
        rec_bc = [None] * NC

        def normalize(c):
            # broadcast the bf16 den row (OT row 64) to 64 partitions via a
            # K=1 matmul, take the fast reciprocal on all 64 partitions at
            # once (FD-bound, one op), then scale OT in place.
            sl = slice(c * NQ, (c + 1) * NQ)
            rb = rbpool.tile([64, 2, NQ], F32, tag="rb")
            for h in range(2):
                R = rpool.tile([64, NQ], F32, tag="R")
                nc.tensor.matmul(R[:], ones_row[64:65, :], OT[h][64:65, sl],
                                 start=True, stop=True)
                nc.vector.reciprocal_approx_fast(rb[:, h, :], R[:])
            for h in range(2):
                nc.vector.tensor_mul(OT[h][0:64, sl],
                                     OT[h][0:64, sl], rb[:, h, :])
            rec_bc[c] = rb

        normalize(0)
        normalize(1)
        for c in range(NC):
            if c + 2 < NC:
                normalize(c + 2)
            for u in range(4 * c, 4 * c + 4):
                ops = ospool.tile([128, 2 * NQ], F32, tag="os")
                for half in range(2):
                    osl = slice(half * NQ, (half + 1) * NQ)
                    nc.tensor.matmul(ops[:, osl],
                                     OT[0][0:64, u * 128:(u + 1) * 128],
                                     wo0[:, osl], start=True, stop=False)
                    nc.tensor.matmul(ops[:, osl],
                                     OT[1][0:64, u * 128:(u + 1) * 128],
                                     wo1[:, osl], start=False, stop=True)
                osb = drpool.tile([128, 2 * NQ], BF16, tag="dr")
                # alternate engines so the drain isn't serialized on one
                if u % 2 == 0:
                    nc.scalar.copy(osb[:], ops[:])
                else:
                    nc.vector.tensor_copy(osb[:], ops[:])
                nc.sync.dma_start(out[u * 128:(u + 1) * 128, :], osb[:])
            rec_bc[c] = None


def build_program():
    nc = bacc.Bacc("TRN2", target_bir_lowering=False, debug=False,
                   enable_asserts=False, num_devices=N_CORES)
    aps = {}
    specs = [
        ("xT", (D, TOK), BF16), ("wq", (D, 128), BF16), ("wk", (D, 128), BF16),
        ("wv", (D, 128), BF16), ("bq", (128, 1), F32), ("bk", (128, 1), F32),
        ("bv", (128, 1), F32), ("wo", (128, D), BF16),
        ("EB0", (NQC * 4 * 128, 2048), BF16), ("EB1", (NQC * 4 * 128, 2048), BF16),
    ]
    for name, shape, dt in specs:
        aps[name] = nc.dram_tensor(name, shape, dt, kind="ExternalInput").ap()
    aps["out"] = nc.dram_tensor("out", (TOK, D), BF16,
                                kind="ExternalOutput").ap()
    with tile.TileContext(nc) as tc:
        with ExitStack() as ctx:
            _body(ctx, tc, aps)
    nc.compile()
    return nc


def _get_nc():
    global _CACHED_NC
    if _CACHED_NC is None:
        _CACHED_NC = build_program()
    return _CACHED_NC


def _host_prep(x, lag, wq, bq, wk, bk, wv, bv, wo, bo, lag_bias):
    x = np.asarray(x, dtype=np.float32)
    lag = np.asarray(lag).astype(np.int64)
    xT = np.ascontiguousarray(
        x.reshape(TOK, D).T.astype(ml_dtypes.bfloat16))
    ld = np.abs(lag[:, None] - lag[None, :]).astype(np.int64)
    lag_bias = np.asarray(lag_bias, dtype=np.float32)
    exp_lb = np.exp(lag_bias).astype(np.float32)
    scale = np.float32(1.0 / np.sqrt(DK))
    wq = np.asarray(wq, dtype=np.float32) * scale
    bq = np.asarray(bq, dtype=np.float32) * scale
    in_maps = []
    for c in range(N_CORES):
        sl = slice(c * 128, (c + 1) * 128)
        cm = {
            "xT": xT,
            "wq": np.ascontiguousarray(wq[:, sl].astype(ml_dtypes.bfloat16)),
            "wk": np.ascontiguousarray(
                np.asarray(wk, np.float32)[:, sl].astype(ml_dtypes.bfloat16)),
            "wv": np.ascontiguousarray(
                np.asarray(wv, np.float32)[:, sl].astype(ml_dtypes.bfloat16)),
            "bq": np.ascontiguousarray(bq[sl].reshape(128, 1)),
            "bk": np.ascontiguousarray(
                np.asarray(bk, np.float32)[sl].reshape(128, 1)),
            "bv": np.ascontiguousarray(
                np.asarray(bv, np.float32)[sl].reshape(128, 1)),
            "wo": np.ascontiguousarray(
                np.asarray(wo, np.float32)[sl, :].astype(ml_dtypes.bfloat16)),
        }
        for hh in range(2):
            # exp(bias) gathered, then pre-tiled so each (qc, jq) DMA
            # reads [128, 4KB-contiguous-per-partition]:
            #   row (qc*4+jq)*128 + p, col ji*512 + q
            #   maps to bias[k = (jq*4+ji)*128 + p, qpos = qc*512 + q]
            eb = exp_lb[2 * c + hh][ld]                       # (S_k, S_q)
            eb6 = eb.reshape(4, 4, 128, NQC, NQ).transpose(3, 0, 2, 1, 4)
            cm[f"EB{hh}"] = np.ascontiguousarray(
                eb6.reshape(NQC * 4 * 128, 2048).astype(ml_dtypes.bfloat16))
        in_maps.append(cm)
    return in_maps


def kernel(x, lag, wq, bq, wk, bk, wv, bv, wo, bo, lag_bias):
    nc = _get_nc()
    in_maps = _host_prep(x, lag, wq, bq, wk, bk, wv, bv, wo, bo, lag_bias)
    kwargs = {}
    if TRACE:
        kwargs = dict(trace=True, tmpdir=TRACE_DIR)
    res = run_bass_kernel_spmd(nc, in_maps, core_ids=list(range(N_CORES)),
                               **kwargs)
    if TRACE:
        print(f"HW exec time: {res.exec_time_ns} ns")
    total = res.results[0]["out"].astype(np.float32)
    for c in range(1, N_CORES):
        total += res.results[c]["out"].astype(np.float32)
    total += np.asarray(bo, dtype=np.float32)[None, :]
    return total.reshape(B, S, D)


# revision 70
# speedup vs baseline: 1.3770x; 1.0030x over previous
"""MultiHeadSelfAttentionWithLagBias on 8 TRN2 NeuronCores.

Sharding: tensor-parallel over heads — 16 heads / 8 cores = 2 heads per
core. Each core computes QKV projections for its head slice (full x),
attention with the lag bias for its 2 heads over both batch elements,
and a partial output projection (its 128 rows of wo). Host sums the 8
partials and adds bo.

v2 design notes (vs the v1 baseline at ~494us):
  - bias applied MULTIPLICATIVELY after exp: exp(s+b) = exp(s)*exp(b).
    Host precomputes exp(lag_bias)[|lag_i-lag_j|] in bf16, pre-tiled to
    the exact DMA layout (4KB contiguous per partition per transfer).
    This moves the bias op off the f32-PSUM path (DVE 1x, ~1.2us) onto
    a bf16 SBUF*SBUF mul (DVE 2x mode, ~0.6us) downstream of the exp.
  - exp output, bias table, V, and the attention-weight matrix are all
    bf16 (sim rel-err 5.2e-3 vs 2e-2 gate); x and the QKV weights are
    bf16 too (halves the 16.8MB xT stream).
  - attention inner loop is ACT(exp)-bound (~1.0us/iter x 128 iters);
    score+PV matmuls, the bias mul, and the bias DMA all pipeline under
    it via double/triple-buffered pools.
  - output projection + normalize deferred to a post-phase (PSUM banks:
    attention needs all 8: 2x2-bank score tiles + 4x1-bank O accum).
  - softmax denominator via the ones-column trick in the PV matmul
    (row 64 of each O accumulator); reciprocal via the fast DVE approx
    (51 ULP, plenty here) instead of the 4us iterative reciprocal.
"""

import ml_dtypes
import numpy as np
from contextlib import ExitStack

import concourse.bass as bass
import concourse.bacc as bacc
import concourse.mybir as mybir
import concourse.tile as tile
from concourse.bass_utils import run_bass_kernel_spmd
from concourse.masks import make_identity

F32 = mybir.dt.float32
F32R = mybir.dt.float32r
BF16 = mybir.dt.bfloat16
AF = mybir.ActivationFunctionType

N_CORES = 8
B, S, D = 2, 2048, 1024
H, DK = 16, 64
TOK = B * S              # 4096
NQ = 512                 # q-chunk (matmul free dim / PSUM bank)
NQC = S // NQ            # 4 q-chunks per batch
NJ = S // 128            # 16 k-chunks per batch
DCH = D // 128           # 8 contraction chunks

# Set by test.py for profiling; harness leaves these untouched.
TRACE = False
TRACE_DIR = None
DEBUG = False

_CACHED_NC = None


def _body(ctx: ExitStack, tc, aps):
    nc = tc.nc
    xT, wq, wk, wv, bq, bk, bv, wo, EB0, EB1, out = (
        aps["xT"], aps["wq"], aps["wk"], aps["wv"], aps["bq"], aps["bk"],
        aps["bv"], aps["wo"], aps["EB0"], aps["EB1"], aps["out"])
    EBh = [EB0, EB1]

    const = ctx.enter_context(tc.tile_pool(name="const", bufs=1))
    persist = ctx.enter_context(tc.tile_pool(name="persist", bufs=1))

    # ---- constants ----
    w_sb = {}
    for name, ap in (("q", wq), ("k", wk), ("v", wv)):
        t = const.tile([128, DCH, 128], BF16, tag=f"w{name}")
        nc.sync.dma_start(t[:], ap.rearrange("(c p) m -> p c m", p=128))
        w_sb[name] = t
    b_sb = {}
    for name, ap in (("q", bq), ("k", bk), ("v", bv)):
        t = const.tile([128, 1], F32, tag=f"b{name}")
        nc.sync.dma_start(t[:], ap[:])
        b_sb[name] = t
    ident = const.tile([128, 128], F32, tag="id")
    make_identity(nc, ident[:])
    # stationary row of ones at partition 64 for broadcasting the softmax
    # denominator (bf16 to match the OT rhs dtype; partition 64 to match
    # the OT den row's base partition)
    ones_row = const.tile([65, 64], BF16, tag="ones_row")
    nc.vector.memset(ones_row[64:65, :], 1.0)

    # ---- persistent activations ----
    QT = persist.tile([128, TOK], BF16, tag="QT")
    KT = persist.tile([128, TOK], BF16, tag="KT")
    Vb = persist.tile([128, TOK // 128, 130], BF16, tag="Vb")
    OT = [persist.tile([65, TOK], BF16, tag=f"OT{h}", name=f"OT{h}")
          for h in range(2)]

    # ones columns of V_ext (positions 64 and 129 of each 130-stripe);
    # staged via an f32 memset + ACT copy (memset on strided bf16 is
    # unreliable).
    ones_f32 = const.tile([128, 64], F32, tag="ones_f32")
    nc.vector.memset(ones_f32[:], 1.0)
    nc.scalar.copy(
        Vb[:].rearrange("p t (g x) -> p t g x", g=2)[:, :, :, 64:65],
        ones_f32[:].rearrange("p (t g x) -> p t g x", t=TOK // 128, g=2))

    # bias tiles are prefetched from inside the projection loop, so the
    # pool opens early
    ebpool = ctx.enter_context(tc.tile_pool(name="eb", bufs=3))
    ebt_tiles = [None] * 17

    def issue_eb(g):
        t = ebpool.tile([128, 2, 4, NQ], BF16, tag="eb")
        for h in range(2):
            r = g * 128
            nc.sync.dma_start(
                t[:, h],
                EBh[h][r:r + 128, :].rearrange("p (i q) -> p i q", i=4))
        ebt_tiles[g] = t

    # ---- phases 1-2: QKV projections + V transpose (scoped pools) ----
    with tc.tile_pool(name="xin", bufs=3) as xpool, \
         tc.tile_pool(name="vtp", bufs=1) as vtpool, \
         tc.tile_pool(name="pj", bufs=3, space="PSUM") as pjpool, \
         tc.tile_pool(name="pt", bufs=2, space="PSUM") as ptpool:
        VT = vtpool.tile([128, TOK], F32, tag="VT")
        xT_r = xT.rearrange("(c p) n -> p c n", p=128)
        for t in range(TOK // NQ):
            xt = xpool.tile([128, DCH, NQ], BF16, tag="x")
            if t == 0:
                # split the first transfer so the first matmuls can start
                # while the rest of the chunk streams in
                nc.sync.dma_start(xt[:, 0:2, :], xT_r[:, 0:2, 0:NQ])
                nc.sync.dma_start(xt[:, 2:DCH, :], xT_r[:, 2:DCH, 0:NQ])
            else:
                nc.sync.dma_start(xt[:], xT_r[:, :, t * NQ:(t + 1) * NQ])
            for name, dst in (("q", QT), ("k", KT), ("v", VT)):
                ps = pjpool.tile([128, NQ], F32, tag="pj")
                for d in range(DCH):
                    nc.tensor.matmul(ps[:], w_sb[name][:, d, :], xt[:, d, :],
                                     start=(d == 0), stop=(d == DCH - 1))
                nc.vector.tensor_scalar_add(
                    dst[:, t * NQ:(t + 1) * NQ], ps[:], b_sb[name][:])
            # V transpose for this token chunk (4 x 128-tok tiles)
            for u in range(t * 4, t * 4 + 4):
                pt = ptpool.tile([128, 128], F32, tag="pt")
                nc.tensor.transpose(pt[:], VT[:, u * 128:(u + 1) * 128],
                                    ident[:])
                nc.scalar.copy(
                    Vb[:, u, :].rearrange("p (g x) -> p g x", g=2)[:, :, 0:64],
                    pt[:].rearrange("p (g x) -> p g x", g=2))
            if t >= 6:
                issue_eb(t - 6)  # prefetch the first bias tiles

    # ---- phase 3: attention (ACT-bound pipeline) ----
    with tc.tile_pool(name="pr", bufs=3) as prpool, \
         tc.tile_pool(name="pe", bufs=3) as pepool, \
         tc.tile_pool(name="sp", bufs=2, space="PSUM") as spool, \
         tc.tile_pool(name="op", bufs=4, space="PSUM") as opool:

        # O-accumulator drain copies are deferred and dribbled out one per
        # iteration (popped right after each bias-mul) so they never
        # head-block the DVE queue. A batch-b accumulator is re-allocated
        # one qc later, right before its batch's loop — by then its drain
        # has been issued during the preceding iterations (with a safety
        # flush at allocation).
        pending = {0: [], 1: []}

        def drain_O(O_ps, hh, b, qc):
            q0 = b * S + qc * NQ
            sl = slice(q0, q0 + NQ)
            tile_ref = O_ps[hh][b]
            pending[b].append(
                lambda: nc.vector.tensor_copy(OT[hh][:, sl], tile_ref[:]))

        def pop_one():
            for bb in (0, 1):
                if pending[bb]:
                    pending[bb].pop(0)()
                    return

        O_ps = [[None, None], [None, None]]
        for g in range(16):
            qc, jq = divmod(g, 4)
            if g + 1 < 16 and g + 1 >= 2:
                issue_eb(g + 1)
            ebt = ebt_tiles[g]
            for b in range(2):
                if jq == 0:
                    while pending[b]:
                        pending[b].pop(0)()
                    for h in range(2):
                        O_ps[h][b] = opool.tile([65, NQ], F32, tag="O",
                                                name=f"O{h}{b}")
                q0 = b * S + qc * NQ
                for ji in range(4):
                    j = jq * 4 + ji
                    k0 = b * S + j * 128
                    sps = spool.tile([128, 2 * NQ], F32, tag="s")
                    for hh in range(2):
                        nc.tensor.matmul(
                            sps[:, hh * NQ:(hh + 1) * NQ],
                            KT[64 * hh:64 * hh + 64, k0:k0 + 128],
                            QT[64 * hh:64 * hh + 64, q0:q0 + NQ],
                            start=True, stop=True)
                    pr = prpool.tile([128, 2 * NQ], BF16, tag="pr")
                    nc.scalar.activation(pr[:], sps[:], AF.Exp)
                    pe = pepool.tile([128, 2 * NQ], BF16, tag="pe")
                    nc.vector.tensor_mul(
                        pe[:].rearrange("p (h q) -> p h q", h=2),
                        pr[:].rearrange("p (h q) -> p h q", h=2),
                        ebt[:, :, ji, :])
                    pop_one()
                    for hh in range(2):
                        nc.tensor.matmul(
                            O_ps[hh][b][:],
                            Vb[:, b * NJ + j, 65 * hh:65 * hh + 65],
                            pe[:, hh * NQ:(hh + 1) * NQ],
                            start=(j == 0), stop=(j == NJ - 1))
                if jq == 3 and b == 0:
                    for hh in range(2):
                        drain_O(O_ps, hh, 0, qc)
            ebt_tiles[g] = None
            if jq == 3:
                for hh in range(2):
                    drain_O(O_ps, hh, 1, qc)
        for bb in (0, 1):
            while pending[bb]:
                pending[bb].pop(0)()

    # ---- phase 4: normalize (all chunks), then a dense out-proj stream ----
    # wo loaded here, off the startup critical path (fires during attention)
    wo0 = const.tile([64, D], BF16, tag="wo0")
    wo1 = const.tile([64, D], BF16, tag="wo1")
    nc.sync.dma_start(wo0[:], wo[0:64, :])
    nc.sync.dma_start(wo1[:], wo[64:128, :])
    with tc.tile_pool(name="rp", bufs=2, space="PSUM") as rpool, \
         tc.tile_pool(name="rb", bufs=3) as rbpool, \
         tc.tile_pool(name="os", bufs=3, space="PSUM") as ospool, \
         tc.tile_pool(name="dr", bufs=4) as drpool:
        NC = TOK // NQ
        rec_bc = [None] * NC

        def normalize(c):
            # broadcast the bf16 den row (OT row 64) to 64 partitions via a
            # K=1 matmul, take the fast reciprocal on all 64 partitions at
            # once (FD-bound, one op), then scale OT in place.
            sl = slice(c * NQ, (c + 1) * NQ)
            rb = rbpool.tile([64, 2, NQ], F32, tag="rb")
            for h in range(2):
                R = rpool.tile([64, NQ], F32, tag="R")
                nc.tensor.matmul(R[:], ones_row[64:65, :], OT[h][64:65, sl],
                                 start=True, stop=True)
                nc.vector.reciprocal_approx_fast(rb[:, h, :], R[:])
            for h in range(2):
                nc.vector.tensor_mul(OT[h][0:64, sl],
                                     OT[h][0:64, sl], rb[:, h, :])
            rec_bc[c] = rb

        normalize(0)
        normalize(1)
        for c in range(NC):
            if c + 2 < NC:
                normalize(c + 2)
            for u in range(4 * c, 4 * c + 4):
                ops = ospool.tile([128, 2 * NQ], F32, tag="os")
                for half in range(2):
                    osl = slice(half * NQ, (half + 1) * NQ)
                    nc.tensor.matmul(ops[:, osl],
                                     OT[0][0:64, u * 128:(u + 1) * 128],
                                     wo0[:, osl], start=True, stop=False)
                    nc.tensor.matmul(ops[:, osl],
                                     OT[1][0:64, u * 128:(u + 1) * 128],
                                     wo1[:, osl], start=False, stop=True)
                # pace the stream to ~90% PE duty: a fully dense matmul
                # stream trips the firmware utilization throttler (0.5x
                # clock clamp), which costs far more than this bubble does
                nc.tensor.drain()
                osb = drpool.tile([128, 2 * NQ], BF16, tag="dr")
                # alternate engines so the drain isn't serialized on one
                if u % 2 == 0:
                    nc.scalar.copy(osb[:], ops[:])
                else:
                    nc.vector.tensor_copy(osb[:], ops[:])
                nc.sync.dma_start(out[u * 128:(u + 1) * 128, :], osb[:])
            rec_bc[c] = None


def build_program():
    nc = bacc.Bacc("TRN2", target_bir_lowering=False, debug=False,
                   enable_asserts=False, num_devices=N_CORES)
    aps = {}
    specs = [
        ("xT", (D, TOK), BF16), ("wq", (D, 128), BF16), ("wk", (D, 128), BF16),
        ("wv", (D, 128), BF16), ("bq", (128, 1), F32), ("bk", (128, 1), F32),
        ("bv", (128, 1), F32), ("wo", (128, D), BF16),
        ("EB0", (NQC * 4 * 128, 2048), BF16), ("EB1", (NQC * 4 * 128, 2048), BF16),
    ]
    for name, shape, dt in specs:
        aps[name] = nc.dram_tensor(name, shape, dt, kind="ExternalInput").ap()
    aps["out"] = nc.dram_tensor("out", (TOK, D), BF16,
                                kind="ExternalOutput").ap()
    with tile.TileContext(nc) as tc:
        with ExitStack() as ctx:
            _body(ctx, tc, aps)
    nc.compile()
    return nc


def _get_nc():
    global _CACHED_NC
    if _CACHED_NC is None:
        _CACHED_NC = build_program()
    return _CACHED_NC


def _host_prep(x, lag, wq, bq, wk, bk, wv, bv, wo, bo, lag_bias):
    x = np.asarray(x, dtype=np.float32)
    lag = np.asarray(lag).astype(np.int64)
    xT = np.ascontiguousarray(
        x.reshape(TOK, D).T.astype(ml_dtypes.bfloat16))
    ld = np.abs(lag[:, None] - lag[None, :]).astype(np.int64)
    lag_bias = np.asarray(lag_bias, dtype=np.float32)
    exp_lb = np.exp(lag_bias).astype(np.float32)
    scale = np.float32(1.0 / np.sqrt(DK))
    wq = np.asarray(wq, dtype=np.float32) * scale
    bq = np.asarray(bq, dtype=np.float32) * scale
    in_maps = []
    for c in range(N_CORES):
        sl = slice(c * 128, (c + 1) * 128)
        cm = {
            "xT": xT,
            "wq": np.ascontiguousarray(wq[:, sl].astype(ml_dtypes.bfloat16)),
            "wk": np.ascontiguousarray(
                np.asarray(wk, np.float32)[:, sl].astype(ml_dtypes.bfloat16)),
            "wv": np.ascontiguousarray(
                np.asarray(wv, np.float32)[:, sl].astype(ml_dtypes.bfloat16)),
            "bq": np.ascontiguousarray(bq[sl].reshape(128, 1)),
            "bk": np.ascontiguousarray(
                np.asarray(bk, np.float32)[sl].reshape(128, 1)),
            "bv": np.ascontiguousarray(
                np.asarray(bv, np.float32)[sl].reshape(128, 1)),
            "wo": np.ascontiguousarray(
                np.asarray(wo, np.float32)[sl, :].astype(ml_dtypes.bfloat16)),
        }
        for hh in range(2):
            # exp(bias) gathered, then pre-tiled so each (qc, jq) DMA
            # reads [128, 4KB-contiguous-per-partition]:
            #   row (qc*4+jq)*128 + p, col ji*512 + q
            #   maps to bias[k = (jq*4+ji)*128 + p, qpos = qc*512 + q]
            eb = exp_lb[2 * c + hh][ld]                       # (S_k, S_q)
            eb6 = eb.reshape(4, 4, 128, NQC, NQ).transpose(3, 0, 2, 1, 4)
            cm[f"EB{hh}"] = np.ascontiguousarray(
                eb6.reshape(NQC * 4 * 128, 2048).astype(ml_dtypes.bfloat16))
        in_maps.append(cm)
    return in_maps


def kernel(x, lag, wq, bq, wk, bk, wv, bv, wo, bo, lag_bias):
    nc = _get_nc()
    in_maps = _host_prep(x, lag, wq, bq, wk, bk, wv, bv, wo, bo, lag_bias)
    kwargs = {}
    if TRACE:
        kwargs = dict(trace=True, tmpdir=TRACE_DIR)
    res = run_bass_kernel_spmd(nc, in_maps, core_ids=list(range(N_CORES)),
                               **kwargs)
    if TRACE:
        print(f"HW exec time: {res.exec_time_ns} ns")
    total = res.results[0]["out"].astype(np.float32)
    for c in range(1, N_CORES):
        total += res.results[c]["out"].astype(np.float32)
    total += np.asarray(bo, dtype=np.float32)[None, :]
    return total.reshape(B, S, D)
